# revision 1
# baseline (speedup 1.0000x reference)
"""ContentStyleReltLoss kernel for 8 Trainium2 NeuronCores.

Sharding: core k handles (batch b = k//2, query-half ih = k%2).
Each core computes, for its batch b and its 2048 "query" columns i of the
reference distance matrices, against ALL 4096 "key" columns j:

  content partial: sum_{j, i in half} | dS(j) - (x~_j . x^q_i - c~_j . c^q_i) |
     with x^ = x/||x|| per column, S_x(j) = HW - u_x . x^_j, u_x = sum_i x^_i,
     Sinv = 1/S, x~ = x^ * Sinv_x, dS(j) = Sinv_x(j) - Sinv_c(j)
  style partials: sim2(j, i) = s^_j . x^q_i
     m1max(i) = max_j sim2  (j complete on device -> final per (b, i))
     m2part(j) = max_{i in half} sim2  (host maxes the two i-halves)

Device layout: j on partitions (32 tiles of 128), i on free dim.
Matmuls bf16, scalars/accumulators f32.  Host only slices inputs and
combines the 8 cores' small partial outputs.
"""

import numpy as np

B, C, H, W = 4, 256, 64, 64
HW = H * W          # 4096
IQ = HW // 2        # 2048 query columns per core
NCORES = 8
NJT = HW // 128     # 32 j-tiles
NIT = 2             # i-tiles
IT = IQ // NIT      # 1024

_CACHED_NC = None


def _build(repeat=1):
    import concourse.bacc as bacc
    import concourse.tile as tile
    from concourse import mybir, bass_isa
    from concourse.alu_op_type import AluOpType
    from contextlib import ExitStack

    dt = mybir.dt
    AF = mybir.ActivationFunctionType
    AX = mybir.AxisListType

    nc = bacc.Bacc(None)

    xf = nc.dram_tensor("xf", [C, HW], dt.float32, kind="ExternalInput")
    cf = nc.dram_tensor("cf", [C, HW], dt.float32, kind="ExternalInput")
    sf = nc.dram_tensor("sf", [C, HW], dt.float32, kind="ExternalInput")
    xq = nc.dram_tensor("xq", [C, IQ], dt.float32, kind="ExternalInput")
    cq = nc.dram_tensor("cq", [C, IQ], dt.float32, kind="ExternalInput")

    o_csum = nc.dram_tensor("csum", [128, 1], dt.float32, kind="ExternalOutput")
    o_m1 = nc.dram_tensor("m1max", [1, IQ], dt.float32, kind="ExternalOutput")
    o_m2 = nc.dram_tensor("m2part", [128, NJT], dt.float32, kind="ExternalOutput")

    with tile.TileContext(nc) as tc, ExitStack() as top:
        pers = top.enter_context(tc.tile_pool(name="pers", bufs=1))
        for _rep in range(repeat):
            # ---------------- persistent tiles ----------------
            # content lhsT (j side): rows 0:256 = x~, 256:512 = c~
            LC = [pers.tile([128, HW], dt.bfloat16, tag=f"lc{i}", name=f"LC{i}")
                  for i in range(4)]
            # content rhs (i side): rows 0:256 = x^q, 256:512 = -c^q
            RC = [pers.tile([128, IQ], dt.bfloat16, tag=f"rc{i}", name=f"RC{i}")
                  for i in range(4)]
            # style lhsT: s^
            LS = [pers.tile([128, HW], dt.bfloat16, tag=f"ls{i}", name=f"LS{i}")
                  for i in range(2)]
            dSc = pers.tile([128, NJT], dt.float32, tag="dsc", name="dSc")
            csum_slots = pers.tile([128, NJT * NIT], dt.float32, tag="cslot", name="cslot")
            m2slots = pers.tile([128, NJT * NIT], dt.float32, tag="m2slot", name="m2slot")
            m1acc = pers.tile([128, IQ], dt.float32, tag="m1acc", name="m1acc")
            ones_bf = pers.tile([128, 128], dt.bfloat16, tag="ones", name="ones_bf")
            nc.vector.memset(ones_bf[:], 1.0)
            b4096 = pers.tile([128, 1], dt.float32, tag="b4096", name="b4096")
            nc.vector.memset(b4096[:], float(HW))

            sic = {}  # per-column 1/S in j-partition layout, for x and c

            # ---------------- preprocessing ----------------
            with tc.tile_pool(name="pre", bufs=1) as pre, \
                 tc.tile_pool(name="pps", bufs=2, space="PSUM") as pps:

                def process(name, dram, n, kind, dst):
                    """Load [C, n] tensor, normalize columns; kind: 'xc'|'s'|'q'|'qneg'."""
                    nh = n // 2048
                    raw = []
                    for k in range(2):
                        t = pre.tile([128, n], dt.float32, tag=f"raw{k}", bufs=2,
                                     name=f"raw_{name}{k}")
                        nc.sync.dma_start(t[:], dram[k * 128:(k + 1) * 128, :])
                        raw.append(t)
                    # column rnorm = 1/||col||, broadcast on partitions [128, n] f32
                    rn = pre.tile([128, HW], dt.float32, tag="rn", bufs=1,
                                  name=f"rn_{name}")
                    for h in range(nh):
                        hs = slice(h * 2048, (h + 1) * 2048)
                        sq = []
                        for k in range(2):
                            q = pre.tile([128, 2048], dt.bfloat16, tag=f"sqh{k}",
                                         bufs=1, name=f"sq_{name}{h}{k}")
                            nc.gpsimd.tensor_mul(q[:], raw[k][:, hs], raw[k][:, hs])
                            sq.append(q)
                        ns = pps.tile([128, 2048], dt.float32, tag="psbig",
                                      name=f"ns_{name}{h}")
                        for k in range(2):
                            for m in range(4):
                                nc.tensor.matmul(
                                    ns[:, m * 512:(m + 1) * 512],
                                    ones_bf[:],
                                    sq[k][:, m * 512:(m + 1) * 512],
                                    start=(k == 0), stop=(k == 1),
                                )
                        lt = pre.tile([128, 2048], dt.float32, tag="lntmp", bufs=1,
                                      name=f"lt_{name}{h}")
                        nc.scalar.activation(lt[:], ns[:], AF.Ln)
                        nc.scalar.activation(rn[:, hs], lt[:], AF.Exp, scale=-0.5)
                    # normalized columns (bf16) into dst chunks
                    for k in range(2):
                        if kind == "qneg":
                            nc.vector.scalar_tensor_tensor(
                                dst[k][:], raw[k][:], -1.0, rn[:, :n],
                                op0=AluOpType.mult, op1=AluOpType.mult)
                        else:
                            nc.gpsimd.tensor_mul(dst[k][:], raw[k][:], rn[:, :n])

                    if kind != "xc":
                        return

                    hat = dst  # for 'xc', dst are transient hat tiles
                    # u = sum_i hat_i  (ACT Copy + accum per half)
                    uh = [[pre.tile([128, 1], dt.float32, tag=f"u{k}h{h}", bufs=2,
                                    name=f"u_{name}{k}{h}") for h in range(2)]
                          for k in range(2)]
                    for k in range(2):
                        for h in range(2):
                            dump = pre.tile([128, 2048], dt.bfloat16, tag=f"sqh{k}",
                                            bufs=1, name=f"dump_{name}{k}{h}")
                            nc.scalar.activation(
                                dump[:], hat[k][:, h * 2048:(h + 1) * 2048],
                                AF.Copy, accum_out=uh[k][h][:])
                    u = [pre.tile([128, 1], dt.float32, tag=f"uu{k}", bufs=2,
                                  name=f"uu_{name}{k}") for k in range(2)]
                    u_bf = [pre.tile([128, 1], dt.bfloat16, tag=f"ub{k}", bufs=2,
                                     name=f"ub_{name}{k}") for k in range(2)]
                    u_rep = [pre.tile([128, 128], dt.bfloat16, tag=f"ur{k}", bufs=2,
                                      name=f"ur_{name}{k}") for k in range(2)]
                    for k in range(2):
                        nc.vector.tensor_add(u[k][:], uh[k][0][:], uh[k][1][:])
                        nc.vector.tensor_copy(u_bf[k][:], u[k][:])
                        nc.vector.tensor_scalar(u_rep[k][:], ones_bf[:], u[k][:],
                                                None, op0=AluOpType.mult)

                    # Sinv broadcast = exp(-ln(HW - u.hat_j))  [128, HW] f32
                    sinv = pre.tile([128, HW], dt.float32, tag="sinv", bufs=1,
                                    name=f"sinv_{name}")
                    for h in range(2):
                        hs = slice(h * 2048, (h + 1) * 2048)
                        su = pps.tile([128, 2048], dt.float32, tag="psbig",
                                      name=f"su_{name}{h}")
                        for k in range(2):
                            for m in range(4):
                                nc.tensor.matmul(
                                    su[:, m * 512:(m + 1) * 512],
                                    u_rep[k][:],
                                    hat[k][:, h * 2048 + m * 512:h * 2048 + (m + 1) * 512],
                                    start=(k == 0), stop=(k == 1),
                                )
                        lt = pre.tile([128, 2048], dt.float32, tag="lntmp", bufs=1,
                                      name=f"lts_{name}{h}")
                        nc.scalar.activation(lt[:], su[:], AF.Ln,
                                             bias=b4096[:], scale=-1.0)
                        nc.scalar.activation(sinv[:, hs], lt[:], AF.Exp, scale=-1.0)

                    # Su_cols [128, NJT] (j on partitions) -> 1/S exact
                    suc = pps.tile([128, NJT], dt.float32, tag="psbig",
                                   name=f"suc_{name}")
                    for jt in range(NJT):
                        for k in range(2):
                            nc.tensor.matmul(
                                suc[:, jt:jt + 1],
                                hat[k][:, jt * 128:(jt + 1) * 128],
                                u_bf[k][:],
                                start=(k == 0), stop=(k == 1),
                            )
                    scol = pre.tile([128, NJT], dt.float32, tag="scol", bufs=2,
                                    name=f"scol_{name}")
                    nc.vector.tensor_scalar(scol[:], suc[:], -1.0, float(HW),
                                            op0=AluOpType.mult, op1=AluOpType.add)
                    s_ic = pers.tile([128, NJT], dt.float32, tag=f"sic_{name}",
                                     name=f"sic_{name}")
                    nc.vector.reciprocal(s_ic[:], scol[:])
                    sic[name] = s_ic

                    # x~ = hat * Sinv into LC rows (mixed bf16*f32 -> bf16)
                    off = 0 if name == "x" else 2
                    for k in range(2):
                        nc.vector.tensor_mul(LC[off + k][:], hat[k][:], sinv[:])

                hat_x = [pre.tile([128, HW], dt.bfloat16, tag=f"hat{k}", bufs=1,
                                  name=f"hatx{k}") for k in range(2)]
                process("x", xf, HW, "xc", hat_x)
                hat_c = [pre.tile([128, HW], dt.bfloat16, tag=f"hat{k}", bufs=1,
                                  name=f"hatc{k}") for k in range(2)]
                process("c", cf, HW, "xc", hat_c)
                process("s", sf, HW, "s", LS)
                process("qx", xq, IQ, "q", [RC[0], RC[1]])
                process("qc", cq, IQ, "qneg", [RC[2], RC[3]])

                nc.vector.tensor_sub(dSc[:], sic["x"][:], sic["c"][:])

            # ---------------- main loop ----------------
            with tc.tile_pool(name="cps", bufs=2, space="PSUM") as cps, \
                 tc.tile_pool(name="sps", bufs=2, space="PSUM") as sps, \
                 tc.tile_pool(name="dmp", bufs=2) as dmp:
                for jt in range(NJT):
                    js = slice(jt * 128, (jt + 1) * 128)
                    for it in range(NIT):
                        idx = jt * NIT + it
                        # content: G' = x~^T x^q - c~^T c^q   (K = 512)
                        psG = cps.tile([128, IT], dt.float32, tag="psG",
                                       name=f"psG{idx}")
                        for k in range(4):
                            for m in range(2):
                                nc.tensor.matmul(
                                    psG[:, m * 512:(m + 1) * 512],
                                    LC[k][:, js],
                                    RC[k][:, it * IT + m * 512:it * IT + (m + 1) * 512],
                                    start=(k == 0), stop=(k == 3),
                                )
                        dump = dmp.tile([128, IT], dt.bfloat16, tag="adump",
                                        name=f"adump{idx}")
                        nc.scalar.activation(
                            dump[:], psG[:], AF.Abs,
                            bias=dSc[:, jt:jt + 1], scale=-1.0,
                            accum_out=csum_slots[:, idx:idx + 1],
                        )
                        # style: sim2 = s^^T x^q   (K = 256)
                        psS = sps.tile([128, IT], dt.float32, tag="psS",
                                       name=f"psS{idx}")
                        for k in range(2):
                            for m in range(2):
                                nc.tensor.matmul(
                                    psS[:, m * 512:(m + 1) * 512],
                                    LS[k][:, js],
                                    RC[k][:, it * IT + m * 512:it * IT + (m + 1) * 512],
                                    start=(k == 0), stop=(k == 1),
                                )
                        nc.vector.reduce_max(m2slots[:, idx:idx + 1], psS[:], axis=AX.X)
                        sl = m1acc[:, it * IT:(it + 1) * IT]
                        if jt == 0:
                            nc.vector.tensor_copy(sl, psS[:])
                        else:
                            nc.vector.tensor_max(sl, sl, psS[:])

                # ---------------- finishers ----------------
                csum = dmp.tile([128, 1], dt.float32, tag="csum", name="csum_f")
                nc.vector.reduce_sum(csum[:], csum_slots[:], axis=AX.X)
                nc.sync.dma_start(o_csum[:], csum[:])

                m2p = dmp.tile([128, NJT], dt.float32, tag="m2p", name="m2p")
                m2v = m2slots[:].rearrange("p (j t) -> p j t", t=NIT)
                nc.vector.tensor_max(m2p[:], m2v[:, :, 0], m2v[:, :, 1])
                nc.sync.dma_start(o_m2[:], m2p[:])

                m1r = dmp.tile([128, IQ], dt.float32, tag="m1r", name="m1r")
                nc.gpsimd.partition_all_reduce(
                    m1r[:], m1acc[:], channels=128,
                    reduce_op=bass_isa.ReduceOp.max)
                nc.sync.dma_start(o_m1[:], m1r[0:1, :])

    nc.finalize()
    return nc


def _get_nc():
    global _CACHED_NC
    if _CACHED_NC is None:
        import os
        _CACHED_NC = _build(repeat=int(os.environ.get("KREPEAT", "1")))
    return _CACHED_NC


_RUNNER = None


def _get_runner():
    """Compile the 8-core PJRT executable once; returns run(in_maps)->results.

    Mirrors concourse.bass2jax.run_bass_via_pjrt but caches the jitted
    executable so repeated kernel() calls only pay device execution.
    """
    global _RUNNER
    if _RUNNER is not None:
        return _RUNNER
    import jax
    import numpy as _np
    from jax.sharding import Mesh, PartitionSpec
    from jax.experimental.shard_map import shard_map
    from concourse import mybir, bass2jax
    from concourse.bass2jax import _bass_exec_p, partition_id_tensor

    bass2jax.install_neuronx_cc_hook()
    nc = _get_nc()
    partition_name = (nc.partition_id_tensor.name
                      if nc.partition_id_tensor else None)

    in_names, out_names, out_avals, zero_outs = [], [], [], []
    for alloc in nc.m.functions[0].allocations:
        if not isinstance(alloc, mybir.MemoryLocationSet):
            continue
        name = alloc.memorylocations[0].name
        if alloc.kind == "ExternalInput":
            if name != partition_name:
                in_names.append(name)
        elif alloc.kind == "ExternalOutput":
            out_names.append(name)
            shape = tuple(alloc.tensor_shape)
            dtype = mybir.dt.np(alloc.dtype)
            out_avals.append(jax.core.ShapedArray(shape, dtype))
            zero_outs.append(_np.zeros((NCORES * shape[0], *shape[1:]), dtype))
    n_params = len(in_names)
    n_outs = len(out_avals)
    all_names = list(in_names) + list(out_names)
    if partition_name is not None:
        all_names.append(partition_name)
    donate = tuple(range(n_params, n_params + n_outs))

    def _body(*args):
        operands = list(args)
        if partition_name is not None:
            operands.append(partition_id_tensor())
        return tuple(_bass_exec_p.bind(
            *operands,
            out_avals=tuple(out_avals),
            in_names=tuple(all_names),
            out_names=tuple(out_names),
            lowering_input_output_aliases=(),
            sim_require_finite=True,
            sim_require_nnan=True,
            nc=nc,
        ))

    devices = jax.devices()[:NCORES]
    mesh = Mesh(_np.asarray(devices), ("core",))
    sharded = jax.jit(
        shard_map(_body, mesh=mesh,
                  in_specs=(PartitionSpec("core"),) * (n_params + n_outs),
                  out_specs=(PartitionSpec("core"),) * n_outs,
                  check_rep=False),
        donate_argnums=donate, keep_unused=True,
    )

    def prepare(in_maps):
        """Stage concatenated inputs onto the devices once (for timing)."""
        from jax.sharding import NamedSharding
        sh = NamedSharding(mesh, PartitionSpec("core"))
        concat_in = [
            _np.concatenate([in_maps[c][nm] for c in range(NCORES)], axis=0)
            for nm in in_names
        ]
        return [jax.device_put(a, sh) for a in concat_in]

    def exec_prepared(staged):
        out_arrs = sharded(*staged, *zero_outs)
        jax.block_until_ready(out_arrs)
        return out_arrs

    def run(in_maps):
        concat_in = [
            _np.concatenate([in_maps[c][nm] for c in range(NCORES)], axis=0)
            for nm in in_names
        ]
        out_arrs = sharded(*concat_in, *zero_outs)
        jax.block_until_ready(out_arrs)
        return [
            {nm: _np.asarray(out_arrs[i]).reshape(NCORES, *out_avals[i].shape)[c]
             for i, nm in enumerate(out_names)}
            for c in range(NCORES)
        ]

    run.prepare = prepare
    run.exec_prepared = exec_prepared
    _RUNNER = run
    return run


def _make_in_maps(x_feat, c_feat, s_feat):
    x = np.asarray(x_feat, dtype=np.float32).reshape(B, C, HW)
    c = np.asarray(c_feat, dtype=np.float32).reshape(B, C, HW)
    s = np.asarray(s_feat, dtype=np.float32).reshape(B, C, HW)
    in_maps = []
    for k in range(NCORES):
        b, ih = k // 2, k % 2
        sl = slice(ih * IQ, (ih + 1) * IQ)
        in_maps.append({
            "xf": np.ascontiguousarray(x[b]),
            "cf": np.ascontiguousarray(c[b]),
            "sf": np.ascontiguousarray(s[b]),
            "xq": np.ascontiguousarray(x[b][:, sl]),
            "cq": np.ascontiguousarray(c[b][:, sl]),
        })
    return in_maps


def kernel(x_feat, c_feat, s_feat):
    outs = _get_runner()(_make_in_maps(x_feat, c_feat, s_feat))

    total = sum(float(r["csum"].sum()) for r in outs)
    content = total / (B * HW)

    m1vals = 1.0 - np.concatenate([r["m1max"][0] for r in outs])
    m1mean = float(m1vals.mean())
    m2mean = 0.0
    for b_ in range(B):
        mx = np.maximum(outs[2 * b_]["m2part"], outs[2 * b_ + 1]["m2part"])
        m2mean += float((1.0 - mx).mean())
    m2mean /= B
    style = max(m1mean, m2mean)

    return (np.float32(content), np.float32(style))



# revision 3
# speedup vs baseline: 21.2864x; 21.2864x over previous
"""ContentStyleReltLoss kernel for 8 Trainium2 NeuronCores.

Sharding: core k handles (batch b = k//2, query-half ih = k%2).
Each core computes, for its batch b and its 2048 "query" columns i of the
reference distance matrices, against ALL 4096 "key" columns j:

  content partial: sum_{j, i in half} | dS(j) - (x~_j . x^q_i - c~_j . c^q_i) |
     with x^ = x/||x|| per column, S_x(j) = HW - u_x . x^_j, u_x = sum_i x^_i,
     Sinv = 1/S, x~ = x^ * Sinv_x, dS(j) = Sinv_x(j) - Sinv_c(j)
  style partials: sim2(j, i) = s^_j . x^q_i
     m1max(i) = max_j sim2  (j complete on device -> final per (b, i))
     m2part(j) = max_{i in half} sim2  (host maxes the two i-halves)

Device layout: j on partitions (32 tiles of 128), i on free dim.
Matmuls bf16, scalars/accumulators f32.  Host only slices inputs and
combines the 8 cores' small partial outputs.
"""

import numpy as np

B, C, H, W = 4, 256, 64, 64
HW = H * W          # 4096
IQ = HW // 2        # 2048 query columns per core
NCORES = 8
NJT = HW // 128     # 32 j-tiles
NIT = 2             # i-tiles
IT = IQ // NIT      # 1024

_CACHED_NC = None


def _build(repeat=1):
    import concourse.bacc as bacc
    import concourse.tile as tile
    from concourse import mybir, bass_isa
    from concourse.alu_op_type import AluOpType
    from contextlib import ExitStack

    dt = mybir.dt
    AF = mybir.ActivationFunctionType
    AX = mybir.AxisListType

    nc = bacc.Bacc(None)

    xf = nc.dram_tensor("xf", [C, HW], dt.float32, kind="ExternalInput")
    cf = nc.dram_tensor("cf", [C, HW], dt.float32, kind="ExternalInput")
    sf = nc.dram_tensor("sf", [C, HW], dt.float32, kind="ExternalInput")
    xq = nc.dram_tensor("xq", [C, IQ], dt.float32, kind="ExternalInput")
    cq = nc.dram_tensor("cq", [C, IQ], dt.float32, kind="ExternalInput")

    o_csum = nc.dram_tensor("csum", [128, 1], dt.float32, kind="ExternalOutput")
    o_m1 = nc.dram_tensor("m1max", [1, IQ], dt.float32, kind="ExternalOutput")
    o_m2 = nc.dram_tensor("m2part", [128, NJT], dt.float32, kind="ExternalOutput")

    with tile.TileContext(nc) as tc, ExitStack() as top:
        pers = top.enter_context(tc.tile_pool(name="pers", bufs=1))
        for _rep in range(repeat):
            # ---------------- persistent tiles ----------------
            # content lhsT (j side): rows 0:256 = x~, 256:512 = c~
            LC = [pers.tile([128, HW], dt.bfloat16, tag=f"lc{i}", name=f"LC{i}")
                  for i in range(4)]
            # content rhs (i side): rows 0:256 = x^q, 256:512 = -c^q
            RC = [pers.tile([128, IQ], dt.bfloat16, tag=f"rc{i}", name=f"RC{i}")
                  for i in range(4)]
            # style lhsT: s^
            LS = [pers.tile([128, HW], dt.bfloat16, tag=f"ls{i}", name=f"LS{i}")
                  for i in range(2)]
            dSc = pers.tile([128, NJT], dt.float32, tag="dsc", name="dSc")
            csum_slots = pers.tile([128, NJT * NIT], dt.float32, tag="cslot", name="cslot")
            m2slots = pers.tile([128, NJT * NIT], dt.float32, tag="m2slot", name="m2slot")
            m1acc = pers.tile([128, IQ], dt.float32, tag="m1acc", name="m1acc")
            ones_bf = pers.tile([128, 128], dt.bfloat16, tag="ones", name="ones_bf")
            nc.vector.memset(ones_bf[:], 1.0)
            b4096 = pers.tile([128, 1], dt.float32, tag="b4096", name="b4096")
            nc.vector.memset(b4096[:], float(HW))

            sic = {}  # per-column 1/S in j-partition layout, for x and c

            # ---------------- preprocessing ----------------
            with tc.tile_pool(name="pre", bufs=1) as pre, \
                 tc.tile_pool(name="pps", bufs=2, space="PSUM") as pps:

                def process(name, dram, n, kind, dst):
                    """Load [C, n] tensor, normalize columns; kind: 'xc'|'s'|'q'|'qneg'."""
                    nh = n // 2048
                    raw = []
                    for k in range(2):
                        t = pre.tile([128, n], dt.float32, tag=f"raw{k}", bufs=2,
                                     name=f"raw_{name}{k}")
                        nc.sync.dma_start(t[:], dram[k * 128:(k + 1) * 128, :])
                        raw.append(t)
                    # column rnorm = 1/||col||, broadcast on partitions [128, n] f32
                    rn = pre.tile([128, HW], dt.float32, tag="rn", bufs=1,
                                  name=f"rn_{name}")
                    for h in range(nh):
                        hs = slice(h * 2048, (h + 1) * 2048)
                        sq = []
                        for k in range(2):
                            q = pre.tile([128, 2048], dt.bfloat16, tag=f"sqh{k}",
                                         bufs=1, name=f"sq_{name}{h}{k}")
                            nc.gpsimd.tensor_mul(q[:], raw[k][:, hs], raw[k][:, hs])
                            sq.append(q)
                        ns = pps.tile([128, 2048], dt.float32, tag="psbig",
                                      name=f"ns_{name}{h}")
                        for k in range(2):
                            for m in range(4):
                                nc.tensor.matmul(
                                    ns[:, m * 512:(m + 1) * 512],
                                    ones_bf[:],
                                    sq[k][:, m * 512:(m + 1) * 512],
                                    start=(k == 0), stop=(k == 1),
                                )
                        lt = pre.tile([128, 2048], dt.float32, tag="lntmp", bufs=1,
                                      name=f"lt_{name}{h}")
                        nc.scalar.activation(lt[:], ns[:], AF.Ln)
                        nc.scalar.activation(rn[:, hs], lt[:], AF.Exp, scale=-0.5)
                    # normalized columns (bf16) into dst chunks
                    for k in range(2):
                        if kind == "qneg":
                            nc.vector.scalar_tensor_tensor(
                                dst[k][:], raw[k][:], -1.0, rn[:, :n],
                                op0=AluOpType.mult, op1=AluOpType.mult)
                        else:
                            nc.gpsimd.tensor_mul(dst[k][:], raw[k][:], rn[:, :n])

                    if kind != "xc":
                        return

                    hat = dst  # for 'xc', dst are transient hat tiles
                    # u = sum_i hat_i  (ACT Copy + accum per half)
                    uh = [[pre.tile([128, 1], dt.float32, tag=f"u{k}h{h}", bufs=2,
                                    name=f"u_{name}{k}{h}") for h in range(2)]
                          for k in range(2)]
                    for k in range(2):
                        for h in range(2):
                            dump = pre.tile([128, 2048], dt.bfloat16, tag=f"sqh{k}",
                                            bufs=1, name=f"dump_{name}{k}{h}")
                            nc.scalar.activation(
                                dump[:], hat[k][:, h * 2048:(h + 1) * 2048],
                                AF.Copy, accum_out=uh[k][h][:])
                    u = [pre.tile([128, 1], dt.float32, tag=f"uu{k}", bufs=2,
                                  name=f"uu_{name}{k}") for k in range(2)]
                    u_bf = [pre.tile([128, 1], dt.bfloat16, tag=f"ub{k}", bufs=2,
                                     name=f"ub_{name}{k}") for k in range(2)]
                    u_rep = [pre.tile([128, 128], dt.bfloat16, tag=f"ur{k}", bufs=2,
                                      name=f"ur_{name}{k}") for k in range(2)]
                    for k in range(2):
                        nc.vector.tensor_add(u[k][:], uh[k][0][:], uh[k][1][:])
                        nc.vector.tensor_copy(u_bf[k][:], u[k][:])
                        nc.vector.tensor_scalar(u_rep[k][:], ones_bf[:], u[k][:],
                                                None, op0=AluOpType.mult)

                    # Sinv broadcast = exp(-ln(HW - u.hat_j))  [128, HW] f32
                    sinv = pre.tile([128, HW], dt.float32, tag="sinv", bufs=1,
                                    name=f"sinv_{name}")
                    for h in range(2):
                        hs = slice(h * 2048, (h + 1) * 2048)
                        su = pps.tile([128, 2048], dt.float32, tag="psbig",
                                      name=f"su_{name}{h}")
                        for k in range(2):
                            for m in range(4):
                                nc.tensor.matmul(
                                    su[:, m * 512:(m + 1) * 512],
                                    u_rep[k][:],
                                    hat[k][:, h * 2048 + m * 512:h * 2048 + (m + 1) * 512],
                                    start=(k == 0), stop=(k == 1),
                                )
                        lt = pre.tile([128, 2048], dt.float32, tag="lntmp", bufs=1,
                                      name=f"lts_{name}{h}")
                        nc.scalar.activation(lt[:], su[:], AF.Ln,
                                             bias=b4096[:], scale=-1.0)
                        nc.scalar.activation(sinv[:, hs], lt[:], AF.Exp, scale=-1.0)

                    # Su_cols [128, NJT] (j on partitions) -> 1/S exact
                    suc = pps.tile([128, NJT], dt.float32, tag="psbig",
                                   name=f"suc_{name}")
                    for jt in range(NJT):
                        for k in range(2):
                            nc.tensor.matmul(
                                suc[:, jt:jt + 1],
                                hat[k][:, jt * 128:(jt + 1) * 128],
                                u_bf[k][:],
                                start=(k == 0), stop=(k == 1),
                            )
                    scol = pre.tile([128, NJT], dt.float32, tag="scol", bufs=2,
                                    name=f"scol_{name}")
                    nc.vector.tensor_scalar(scol[:], suc[:], -1.0, float(HW),
                                            op0=AluOpType.mult, op1=AluOpType.add)
                    s_ic = pers.tile([128, NJT], dt.float32, tag=f"sic_{name}",
                                     name=f"sic_{name}")
                    nc.vector.reciprocal(s_ic[:], scol[:])
                    sic[name] = s_ic

                    # x~ = hat * Sinv into LC rows (mixed bf16*f32 -> bf16)
                    off = 0 if name == "x" else 2
                    for k in range(2):
                        nc.vector.tensor_mul(LC[off + k][:], hat[k][:], sinv[:])

                hat_x = [pre.tile([128, HW], dt.bfloat16, tag=f"hat{k}", bufs=1,
                                  name=f"hatx{k}") for k in range(2)]
                process("x", xf, HW, "xc", hat_x)
                hat_c = [pre.tile([128, HW], dt.bfloat16, tag=f"hat{k}", bufs=1,
                                  name=f"hatc{k}") for k in range(2)]
                process("c", cf, HW, "xc", hat_c)
                process("s", sf, HW, "s", LS)
                process("qx", xq, IQ, "q", [RC[0], RC[1]])
                process("qc", cq, IQ, "qneg", [RC[2], RC[3]])

                nc.vector.tensor_sub(dSc[:], sic["x"][:], sic["c"][:])

            # ---------------- main loop ----------------
            with tc.tile_pool(name="cps", bufs=2, space="PSUM") as cps, \
                 tc.tile_pool(name="sps", bufs=2, space="PSUM") as sps, \
                 tc.tile_pool(name="dmp", bufs=2) as dmp:
                for jt in range(NJT):
                    js = slice(jt * 128, (jt + 1) * 128)
                    for it in range(NIT):
                        idx = jt * NIT + it
                        # content: G' = x~^T x^q - c~^T c^q   (K = 512)
                        psG = cps.tile([128, IT], dt.float32, tag="psG",
                                       name=f"psG{idx}")
                        for k in range(4):
                            for m in range(2):
                                nc.tensor.matmul(
                                    psG[:, m * 512:(m + 1) * 512],
                                    LC[k][:, js],
                                    RC[k][:, it * IT + m * 512:it * IT + (m + 1) * 512],
                                    start=(k == 0), stop=(k == 3),
                                )
                        dump = dmp.tile([128, IT], dt.bfloat16, tag="adump",
                                        name=f"adump{idx}")
                        nc.scalar.activation(
                            dump[:], psG[:], AF.Abs,
                            bias=dSc[:, jt:jt + 1], scale=-1.0,
                            accum_out=csum_slots[:, idx:idx + 1],
                        )
                        # style: sim2 = s^^T x^q   (K = 256)
                        psS = sps.tile([128, IT], dt.float32, tag="psS",
                                       name=f"psS{idx}")
                        for k in range(2):
                            for m in range(2):
                                nc.tensor.matmul(
                                    psS[:, m * 512:(m + 1) * 512],
                                    LS[k][:, js],
                                    RC[k][:, it * IT + m * 512:it * IT + (m + 1) * 512],
                                    start=(k == 0), stop=(k == 1),
                                )
                        nc.vector.reduce_max(m2slots[:, idx:idx + 1], psS[:], axis=AX.X)
                        sl = m1acc[:, it * IT:(it + 1) * IT]
                        if jt == 0:
                            nc.vector.tensor_copy(sl, psS[:])
                        else:
                            nc.vector.tensor_max(sl, sl, psS[:])

                # ---------------- finishers ----------------
                csum = dmp.tile([128, 1], dt.float32, tag="csum", name="csum_f")
                nc.vector.reduce_sum(csum[:], csum_slots[:], axis=AX.X)
                nc.sync.dma_start(o_csum[:], csum[:])

                m2p = dmp.tile([128, NJT], dt.float32, tag="m2p", name="m2p")
                m2v = m2slots[:].rearrange("p (j t) -> p j t", t=NIT)
                nc.vector.tensor_max(m2p[:], m2v[:, :, 0], m2v[:, :, 1])
                nc.sync.dma_start(o_m2[:], m2p[:])

                m1r = dmp.tile([128, IQ], dt.float32, tag="m1r", name="m1r")
                nc.gpsimd.partition_all_reduce(
                    m1r[:], m1acc[:], channels=128,
                    reduce_op=bass_isa.ReduceOp.max)
                nc.sync.dma_start(o_m1[:], m1r[0:1, :])

    nc.finalize()
    return nc


def _get_nc():
    global _CACHED_NC
    if _CACHED_NC is None:
        import os
        _CACHED_NC = _build(repeat=int(os.environ.get("KREPEAT", "1")))
    return _CACHED_NC


_RUNNER = None


def _get_runner():
    """Compile the 8-core PJRT executable once; returns run(in_maps)->results.

    Mirrors concourse.bass2jax.run_bass_via_pjrt but caches the jitted
    executable so repeated kernel() calls only pay device execution.
    """
    global _RUNNER
    if _RUNNER is not None:
        return _RUNNER
    import jax
    import numpy as _np
    from jax.sharding import Mesh, PartitionSpec
    from jax.experimental.shard_map import shard_map
    from concourse import mybir, bass2jax
    from concourse.bass2jax import _bass_exec_p, partition_id_tensor

    bass2jax.install_neuronx_cc_hook()
    nc = _get_nc()
    partition_name = (nc.partition_id_tensor.name
                      if nc.partition_id_tensor else None)

    in_names, out_names, out_avals, zero_outs = [], [], [], []
    for alloc in nc.m.functions[0].allocations:
        if not isinstance(alloc, mybir.MemoryLocationSet):
            continue
        name = alloc.memorylocations[0].name
        if alloc.kind == "ExternalInput":
            if name != partition_name:
                in_names.append(name)
        elif alloc.kind == "ExternalOutput":
            out_names.append(name)
            shape = tuple(alloc.tensor_shape)
            dtype = mybir.dt.np(alloc.dtype)
            out_avals.append(jax.core.ShapedArray(shape, dtype))
            zero_outs.append(_np.zeros((NCORES * shape[0], *shape[1:]), dtype))
    n_params = len(in_names)
    n_outs = len(out_avals)
    all_names = list(in_names) + list(out_names)
    if partition_name is not None:
        all_names.append(partition_name)

    def _body(*args):
        operands = list(args)
        if partition_name is not None:
            operands.append(partition_id_tensor())
        return tuple(_bass_exec_p.bind(
            *operands,
            out_avals=tuple(out_avals),
            in_names=tuple(all_names),
            out_names=tuple(out_names),
            lowering_input_output_aliases=(),
            sim_require_finite=True,
            sim_require_nnan=True,
            nc=nc,
        ))

    devices = jax.devices()[:NCORES]
    mesh = Mesh(_np.asarray(devices), ("core",))
    sharded = jax.jit(
        shard_map(_body, mesh=mesh,
                  in_specs=(PartitionSpec("core"),) * (n_params + n_outs),
                  out_specs=(PartitionSpec("core"),) * n_outs,
                  check_rep=False),
        keep_unused=True,
    )

    from jax.sharding import NamedSharding
    sh = NamedSharding(mesh, PartitionSpec("core"))
    zero_dev = [jax.device_put(a, sh) for a in zero_outs]

    def prepare(in_maps):
        """Stage concatenated inputs onto the devices once (for timing)."""
        concat_in = [
            _np.concatenate([in_maps[c][nm] for c in range(NCORES)], axis=0)
            for nm in in_names
        ]
        return [jax.device_put(a, sh) for a in concat_in]

    def exec_prepared(staged):
        out_arrs = sharded(*staged, *zero_dev)
        jax.block_until_ready(out_arrs)
        return out_arrs

    def exec_async(staged):
        """Dispatch one execution without blocking (pipelined timing)."""
        return sharded(*staged, *zero_dev)

    def run(in_maps):
        concat_in = [
            _np.concatenate([in_maps[c][nm] for c in range(NCORES)], axis=0)
            for nm in in_names
        ]
        out_arrs = sharded(*concat_in, *zero_dev)
        jax.block_until_ready(out_arrs)
        return [
            {nm: _np.asarray(out_arrs[i]).reshape(NCORES, *out_avals[i].shape)[c]
             for i, nm in enumerate(out_names)}
            for c in range(NCORES)
        ]

    run.prepare = prepare
    run.exec_prepared = exec_prepared
    run.exec_async = exec_async
    _RUNNER = run
    return run


def _make_in_maps(x_feat, c_feat, s_feat):
    x = np.asarray(x_feat, dtype=np.float32).reshape(B, C, HW)
    c = np.asarray(c_feat, dtype=np.float32).reshape(B, C, HW)
    s = np.asarray(s_feat, dtype=np.float32).reshape(B, C, HW)
    in_maps = []
    for k in range(NCORES):
        b, ih = k // 2, k % 2
        sl = slice(ih * IQ, (ih + 1) * IQ)
        in_maps.append({
            "xf": np.ascontiguousarray(x[b]),
            "cf": np.ascontiguousarray(c[b]),
            "sf": np.ascontiguousarray(s[b]),
            "xq": np.ascontiguousarray(x[b][:, sl]),
            "cq": np.ascontiguousarray(c[b][:, sl]),
        })
    return in_maps


def kernel(x_feat, c_feat, s_feat):
    outs = _get_runner()(_make_in_maps(x_feat, c_feat, s_feat))

    total = sum(float(r["csum"].sum()) for r in outs)
    content = total / (B * HW)

    m1vals = 1.0 - np.concatenate([r["m1max"][0] for r in outs])
    m1mean = float(m1vals.mean())
    m2mean = 0.0
    for b_ in range(B):
        mx = np.maximum(outs[2 * b_]["m2part"], outs[2 * b_ + 1]["m2part"])
        m2mean += float((1.0 - mx).mean())
    m2mean /= B
    style = max(m1mean, m2mean)

    return (np.float32(content), np.float32(style))



# revision 6
# speedup vs baseline: 25.7389x; 1.2092x over previous
"""ContentStyleReltLoss kernel for 8 Trainium2 NeuronCores.

Sharding: core k handles (batch b = k//2, query-half ih = k%2).
Each core computes, for its batch b and its 2048 "query" columns i of the
reference distance matrices, against ALL 4096 "key" columns j:

  content partial: sum_{j, i in half} | dS(j) - (x~_j . x^q_i - c~_j . c^q_i) |
     with x^ = x/||x|| per column, S_x(j) = HW - u_x . x^_j, u_x = sum_i x^_i,
     Sinv = 1/S, x~ = x^ * Sinv_x, dS(j) = Sinv_x(j) - Sinv_c(j)
  style partials: sim2(j, i) = s^_j . x^q_i
     m1max(i) = max_j sim2  (j complete on device -> final per (b, i))
     m2part(j) = max_{i in half} sim2  (host maxes the two i-halves)

Device layout: j on partitions (32 tiles of 128), i on free dim.
Matmuls bf16, scalars/accumulators f32.  Host only slices inputs and
combines the 8 cores' small partial outputs.
"""

import numpy as np

B, C, H, W = 4, 256, 64, 64
HW = H * W          # 4096
IQ = HW // 2        # 2048 query columns per core
NCORES = 8
NJT = HW // 128     # 32 j-tiles
NIT = 2             # i-tiles
IT = IQ // NIT      # 1024

_CACHED_NC = None


def _build(repeat=1):
    import concourse.bacc as bacc
    import concourse.tile as tile
    from concourse import mybir, bass_isa
    from concourse.alu_op_type import AluOpType
    from contextlib import ExitStack

    dt = mybir.dt
    AF = mybir.ActivationFunctionType
    AX = mybir.AxisListType

    nc = bacc.Bacc(None)

    xf = nc.dram_tensor("xf", [C, HW], dt.float32, kind="ExternalInput")
    cf = nc.dram_tensor("cf", [C, HW], dt.float32, kind="ExternalInput")
    sf = nc.dram_tensor("sf", [C, HW], dt.float32, kind="ExternalInput")
    xq = nc.dram_tensor("xq", [C, IQ], dt.float32, kind="ExternalInput")
    cq = nc.dram_tensor("cq", [C, IQ], dt.float32, kind="ExternalInput")

    o_csum = nc.dram_tensor("csum", [128, 1], dt.float32, kind="ExternalOutput")
    o_m1 = nc.dram_tensor("m1max", [1, IQ], dt.float32, kind="ExternalOutput")
    o_m2 = nc.dram_tensor("m2part", [128, NJT], dt.float32, kind="ExternalOutput")

    with tile.TileContext(nc) as tc, ExitStack() as top:
        pers = top.enter_context(tc.tile_pool(name="pers", bufs=1))
        for _rep in range(repeat):
            # ---------------- persistent tiles ----------------
            # content lhsT (j side): rows 0:256 = x~, 256:512 = c~
            LC = [pers.tile([128, HW], dt.bfloat16, tag=f"lc{i}", name=f"LC{i}")
                  for i in range(4)]
            # content rhs (i side): rows 0:256 = x^q, 256:512 = -c^q
            RC = [pers.tile([128, IQ], dt.bfloat16, tag=f"rc{i}", name=f"RC{i}")
                  for i in range(4)]
            # style lhsT: s^
            LS = [pers.tile([128, HW], dt.bfloat16, tag=f"ls{i}", name=f"LS{i}")
                  for i in range(2)]
            dSc = pers.tile([128, NJT], dt.float32, tag="dsc", name="dSc")
            csum_slots = pers.tile([128, NJT * NIT], dt.float32, tag="cslot", name="cslot")
            m2slots = pers.tile([128, NJT * NIT], dt.float32, tag="m2slot", name="m2slot")
            m1acc = pers.tile([128, IQ], dt.float32, tag="m1acc", name="m1acc")
            ones_bf = pers.tile([128, 128], dt.bfloat16, tag="ones", name="ones_bf")
            nc.vector.memset(ones_bf[:], 1.0)
            b4096 = pers.tile([128, 1], dt.float32, tag="b4096", name="b4096")
            nc.vector.memset(b4096[:], float(HW))

            sic = {}  # per-column 1/S in j-partition layout, for x and c

            # ---------------- preprocessing ----------------
            with tc.tile_pool(name="pre", bufs=1) as pre, \
                 tc.tile_pool(name="pps", bufs=2, space="PSUM") as pps:

                def process(name, dram, n, kind, dst):
                    """Load [C, n] tensor, normalize columns; kind: 'xc'|'s'|'q'|'qneg'."""
                    nh = n // 2048
                    raw = []
                    for k in range(2):
                        t = pre.tile([128, n], dt.float32, tag=f"raw{k}", bufs=2,
                                     name=f"raw_{name}{k}")
                        nc.sync.dma_start(t[:], dram[k * 128:(k + 1) * 128, :])
                        raw.append(t)
                    # column rnorm = 1/||col||, broadcast on partitions [128, n] f32
                    rn = pre.tile([128, HW], dt.float32, tag="rn", bufs=1,
                                  name=f"rn_{name}")
                    for h in range(nh):
                        hs = slice(h * 2048, (h + 1) * 2048)
                        sq = []
                        for k in range(2):
                            q = pre.tile([128, 2048], dt.bfloat16, tag=f"sqh{k}",
                                         bufs=1, name=f"sq_{name}{h}{k}")
                            nc.gpsimd.tensor_mul(q[:], raw[k][:, hs], raw[k][:, hs])
                            sq.append(q)
                        ns = pps.tile([128, 2048], dt.float32, tag="psbig",
                                      name=f"ns_{name}{h}")
                        for k in range(2):
                            for m in range(4):
                                nc.tensor.matmul(
                                    ns[:, m * 512:(m + 1) * 512],
                                    ones_bf[:],
                                    sq[k][:, m * 512:(m + 1) * 512],
                                    start=(k == 0), stop=(k == 1),
                                )
                        lt = pre.tile([128, 2048], dt.float32, tag="lntmp", bufs=1,
                                      name=f"lt_{name}{h}")
                        nc.scalar.activation(lt[:], ns[:], AF.Ln)
                        nc.scalar.activation(rn[:, hs], lt[:], AF.Exp, scale=-0.5)
                    # normalized columns (bf16) into dst chunks
                    for k in range(2):
                        if kind == "qneg":
                            nc.vector.scalar_tensor_tensor(
                                dst[k][:], raw[k][:], -1.0, rn[:, :n],
                                op0=AluOpType.mult, op1=AluOpType.mult)
                        else:
                            nc.gpsimd.tensor_mul(dst[k][:], raw[k][:], rn[:, :n])

                    if kind != "xc":
                        return

                    hat = dst  # for 'xc', dst are transient hat tiles
                    # u = sum_i hat_i  (ACT Copy + accum per half)
                    uh = [[pre.tile([128, 1], dt.float32, tag=f"u{k}h{h}", bufs=2,
                                    name=f"u_{name}{k}{h}") for h in range(2)]
                          for k in range(2)]
                    for k in range(2):
                        for h in range(2):
                            dump = pre.tile([128, 2048], dt.bfloat16, tag=f"sqh{k}",
                                            bufs=1, name=f"dump_{name}{k}{h}")
                            nc.scalar.activation(
                                dump[:], hat[k][:, h * 2048:(h + 1) * 2048],
                                AF.Copy, accum_out=uh[k][h][:])
                    u = [pre.tile([128, 1], dt.float32, tag=f"uu{k}", bufs=2,
                                  name=f"uu_{name}{k}") for k in range(2)]
                    u_bf = [pre.tile([128, 1], dt.bfloat16, tag=f"ub{k}", bufs=2,
                                     name=f"ub_{name}{k}") for k in range(2)]
                    u_rep = [pre.tile([128, 128], dt.bfloat16, tag=f"ur{k}", bufs=2,
                                      name=f"ur_{name}{k}") for k in range(2)]
                    for k in range(2):
                        nc.vector.tensor_add(u[k][:], uh[k][0][:], uh[k][1][:])
                        nc.vector.tensor_copy(u_bf[k][:], u[k][:])
                        nc.vector.tensor_scalar(u_rep[k][:], ones_bf[:], u[k][:],
                                                None, op0=AluOpType.mult)

                    # Sinv broadcast = exp(-ln(HW - u.hat_j))  [128, HW] f32
                    sinv = pre.tile([128, HW], dt.float32, tag="sinv", bufs=1,
                                    name=f"sinv_{name}")
                    for h in range(2):
                        hs = slice(h * 2048, (h + 1) * 2048)
                        su = pps.tile([128, 2048], dt.float32, tag="psbig",
                                      name=f"su_{name}{h}")
                        for k in range(2):
                            for m in range(4):
                                nc.tensor.matmul(
                                    su[:, m * 512:(m + 1) * 512],
                                    u_rep[k][:],
                                    hat[k][:, h * 2048 + m * 512:h * 2048 + (m + 1) * 512],
                                    start=(k == 0), stop=(k == 1),
                                )
                        lt = pre.tile([128, 2048], dt.float32, tag="lntmp", bufs=1,
                                      name=f"lts_{name}{h}")
                        nc.scalar.activation(lt[:], su[:], AF.Ln,
                                             bias=b4096[:], scale=-1.0)
                        nc.scalar.activation(sinv[:, hs], lt[:], AF.Exp, scale=-1.0)

                    # Su_cols [128, NJT] (j on partitions) -> 1/S exact
                    suc = pps.tile([128, NJT], dt.float32, tag="psbig",
                                   name=f"suc_{name}")
                    for jt in range(NJT):
                        for k in range(2):
                            nc.tensor.matmul(
                                suc[:, jt:jt + 1],
                                hat[k][:, jt * 128:(jt + 1) * 128],
                                u_bf[k][:],
                                start=(k == 0), stop=(k == 1),
                            )
                    scol = pre.tile([128, NJT], dt.float32, tag="scol", bufs=2,
                                    name=f"scol_{name}")
                    nc.vector.tensor_scalar(scol[:], suc[:], -1.0, float(HW),
                                            op0=AluOpType.mult, op1=AluOpType.add)
                    s_ic = pers.tile([128, NJT], dt.float32, tag=f"sic_{name}",
                                     name=f"sic_{name}")
                    nc.vector.reciprocal(s_ic[:], scol[:])
                    sic[name] = s_ic

                    # x~ = hat * Sinv into LC rows (mixed bf16*f32 -> bf16)
                    off = 0 if name == "x" else 2
                    for k in range(2):
                        nc.vector.tensor_mul(LC[off + k][:], hat[k][:], sinv[:])

                hat_x = [pre.tile([128, HW], dt.bfloat16, tag=f"hat{k}", bufs=1,
                                  name=f"hatx{k}") for k in range(2)]
                process("x", xf, HW, "xc", hat_x)
                hat_c = [pre.tile([128, HW], dt.bfloat16, tag=f"hat{k}", bufs=1,
                                  name=f"hatc{k}") for k in range(2)]
                process("c", cf, HW, "xc", hat_c)
                process("s", sf, HW, "s", LS)
                process("qx", xq, IQ, "q", [RC[0], RC[1]])
                process("qc", cq, IQ, "qneg", [RC[2], RC[3]])

                nc.vector.tensor_sub(dSc[:], sic["x"][:], sic["c"][:])

            # ---------------- main loop ----------------
            with tc.tile_pool(name="cps", bufs=2, space="PSUM") as cps, \
                 tc.tile_pool(name="sps", bufs=2, space="PSUM") as sps, \
                 tc.tile_pool(name="dmp", bufs=2) as dmp:
                for jt in range(NJT):
                    js = slice(jt * 128, (jt + 1) * 128)
                    for it in range(NIT):
                        idx = jt * NIT + it
                        # content: G' = x~^T x^q - c~^T c^q   (K = 512)
                        psG = cps.tile([128, IT], dt.float32, tag="psG",
                                       name=f"psG{idx}")
                        for k in range(4):
                            for m in range(2):
                                nc.tensor.matmul(
                                    psG[:, m * 512:(m + 1) * 512],
                                    LC[k][:, js],
                                    RC[k][:, it * IT + m * 512:it * IT + (m + 1) * 512],
                                    start=(k == 0), stop=(k == 3),
                                )
                        dump = dmp.tile([128, IT], dt.bfloat16, tag="adump",
                                        name=f"adump{idx}")
                        nc.scalar.activation(
                            dump[:], psG[:], AF.Abs,
                            bias=dSc[:, jt:jt + 1], scale=-1.0,
                            accum_out=csum_slots[:, idx:idx + 1],
                        )
                        # style: sim2 = s^^T x^q   (K = 256)
                        psS = sps.tile([128, IT], dt.float32, tag="psS",
                                       name=f"psS{idx}")
                        for k in range(2):
                            for m in range(2):
                                nc.tensor.matmul(
                                    psS[:, m * 512:(m + 1) * 512],
                                    LS[k][:, js],
                                    RC[k][:, it * IT + m * 512:it * IT + (m + 1) * 512],
                                    start=(k == 0), stop=(k == 1),
                                )
                        nc.vector.reduce_max(m2slots[:, idx:idx + 1], psS[:], axis=AX.X)
                        sl = m1acc[:, it * IT:(it + 1) * IT]
                        if jt == 0:
                            nc.vector.tensor_copy(sl, psS[:])
                        else:
                            nc.vector.tensor_max(sl, sl, psS[:])

                # ---------------- finishers ----------------
                csum = dmp.tile([128, 1], dt.float32, tag="csum", name="csum_f")
                nc.vector.reduce_sum(csum[:], csum_slots[:], axis=AX.X)
                nc.sync.dma_start(o_csum[:], csum[:])

                m2p = dmp.tile([128, NJT], dt.float32, tag="m2p", name="m2p")
                m2v = m2slots[:].rearrange("p (j t) -> p j t", t=NIT)
                nc.vector.tensor_max(m2p[:], m2v[:, :, 0], m2v[:, :, 1])
                nc.sync.dma_start(o_m2[:], m2p[:])

                m1r = dmp.tile([128, IQ], dt.float32, tag="m1r", name="m1r")
                nc.gpsimd.partition_all_reduce(
                    m1r[:], m1acc[:], channels=128,
                    reduce_op=bass_isa.ReduceOp.max)
                nc.sync.dma_start(o_m1[:], m1r[0:1, :])

    nc.finalize()
    return nc


def _get_nc():
    global _CACHED_NC
    if _CACHED_NC is None:
        import os
        _CACHED_NC = _build(repeat=int(os.environ.get("KREPEAT", "1")))
    return _CACHED_NC


_RUNNER = None


def _get_runner():
    """Compile the 8-core PJRT executable once; returns run(in_maps)->results.

    Mirrors concourse.bass2jax.run_bass_via_pjrt but caches the jitted
    executable so repeated kernel() calls only pay device execution.
    """
    global _RUNNER
    if _RUNNER is not None:
        return _RUNNER
    import jax
    import numpy as _np
    from jax.sharding import Mesh, PartitionSpec
    from jax.experimental.shard_map import shard_map
    from concourse import mybir, bass2jax
    from concourse.bass2jax import (_bass_exec_p, partition_id_tensor,
                                    fast_dispatch_compile)

    bass2jax.install_neuronx_cc_hook()
    nc = _get_nc()
    partition_name = (nc.partition_id_tensor.name
                      if nc.partition_id_tensor else None)

    in_names, out_names, out_avals, zero_outs = [], [], [], []
    in_shapes = []
    for alloc in nc.m.functions[0].allocations:
        if not isinstance(alloc, mybir.MemoryLocationSet):
            continue
        name = alloc.memorylocations[0].name
        if alloc.kind == "ExternalInput":
            if name != partition_name:
                in_names.append(name)
                in_shapes.append((tuple(alloc.tensor_shape),
                                  mybir.dt.np(alloc.dtype)))
        elif alloc.kind == "ExternalOutput":
            out_names.append(name)
            shape = tuple(alloc.tensor_shape)
            dtype = mybir.dt.np(alloc.dtype)
            out_avals.append(jax.core.ShapedArray(shape, dtype))
            zero_outs.append(_np.zeros((NCORES * shape[0], *shape[1:]), dtype))
    n_params = len(in_names)
    n_outs = len(out_avals)
    all_names = list(in_names) + list(out_names)
    if partition_name is not None:
        all_names.append(partition_name)

    def _body(*args):
        operands = list(args)
        if partition_name is not None:
            operands.append(partition_id_tensor())
        return tuple(_bass_exec_p.bind(
            *operands,
            out_avals=tuple(out_avals),
            in_names=tuple(all_names),
            out_names=tuple(out_names),
            lowering_input_output_aliases=(),
            sim_require_finite=True,
            sim_require_nnan=True,
            nc=nc,
        ))

    devices = jax.devices()[:NCORES]
    mesh = Mesh(_np.asarray(devices), ("core",))
    from jax.sharding import NamedSharding
    sh = NamedSharding(mesh, PartitionSpec("core"))

    # AOT-compile with bass_effect suppressed: the default effectful path
    # forces slow Python dispatch with effect tokens on every call; the
    # fast path dispatches through C++ (see bass2jax.fast_dispatch_compile).
    arg_structs = (
        [jax.ShapeDtypeStruct((NCORES * s[0], *s[1:]), dt, sharding=sh)
         for s, dt in in_shapes]
        + [jax.ShapeDtypeStruct((NCORES * a.shape[0], *a.shape[1:]), a.dtype,
                                sharding=sh) for a in out_avals]
    )

    def _compile():
        return jax.jit(
            shard_map(_body, mesh=mesh,
                      in_specs=(PartitionSpec("core"),) * (n_params + n_outs),
                      out_specs=(PartitionSpec("core"),) * n_outs,
                      check_rep=False),
            keep_unused=True,
        ).lower(*arg_structs).compile()

    sharded = fast_dispatch_compile(_compile)
    zero_dev = [jax.device_put(a, sh) for a in zero_outs]

    def prepare(in_maps):
        """Stage concatenated inputs onto the devices once (for timing)."""
        concat_in = [
            _np.concatenate([in_maps[c][nm] for c in range(NCORES)], axis=0)
            for nm in in_names
        ]
        return [jax.device_put(a, sh) for a in concat_in]

    def exec_prepared(staged):
        out_arrs = sharded(*staged, *zero_dev)
        jax.block_until_ready(out_arrs)
        return out_arrs

    def exec_async(staged):
        """Dispatch one execution without blocking (pipelined timing)."""
        return sharded(*staged, *zero_dev)

    def run(in_maps):
        concat_in = [
            jax.device_put(
                _np.concatenate([in_maps[c][nm] for c in range(NCORES)], axis=0),
                sh)
            for nm in in_names
        ]
        out_arrs = sharded(*concat_in, *zero_dev)
        jax.block_until_ready(out_arrs)
        return [
            {nm: _np.asarray(out_arrs[i]).reshape(NCORES, *out_avals[i].shape)[c]
             for i, nm in enumerate(out_names)}
            for c in range(NCORES)
        ]

    run.prepare = prepare
    run.exec_prepared = exec_prepared
    run.exec_async = exec_async
    _RUNNER = run
    return run


def _make_in_maps(x_feat, c_feat, s_feat):
    x = np.asarray(x_feat, dtype=np.float32).reshape(B, C, HW)
    c = np.asarray(c_feat, dtype=np.float32).reshape(B, C, HW)
    s = np.asarray(s_feat, dtype=np.float32).reshape(B, C, HW)
    in_maps = []
    for k in range(NCORES):
        b, ih = k // 2, k % 2
        sl = slice(ih * IQ, (ih + 1) * IQ)
        in_maps.append({
            "xf": np.ascontiguousarray(x[b]),
            "cf": np.ascontiguousarray(c[b]),
            "sf": np.ascontiguousarray(s[b]),
            "xq": np.ascontiguousarray(x[b][:, sl]),
            "cq": np.ascontiguousarray(c[b][:, sl]),
        })
    return in_maps


def kernel(x_feat, c_feat, s_feat):
    outs = _get_runner()(_make_in_maps(x_feat, c_feat, s_feat))

    total = sum(float(r["csum"].sum()) for r in outs)
    content = total / (B * HW)

    m1vals = 1.0 - np.concatenate([r["m1max"][0] for r in outs])
    m1mean = float(m1vals.mean())
    m2mean = 0.0
    for b_ in range(B):
        mx = np.maximum(outs[2 * b_]["m2part"], outs[2 * b_ + 1]["m2part"])
        m2mean += float((1.0 - mx).mean())
    m2mean /= B
    style = max(m1mean, m2mean)

    return (np.float32(content), np.float32(style))



# revision 7
# speedup vs baseline: 114.7854x; 4.4596x over previous
"""ContentStyleReltLoss kernel for 8 Trainium2 NeuronCores.

Sharding: core k handles (batch b = k//2, query-half ih = k%2).
Host rolls the HW columns of x/c/s by -ih*2048 so every core's 2048
"query" columns sit at device columns 0:2048 — all cores run identical
code. Each core computes, for its 2048 query columns i against ALL
4096 "key" columns j:

  content partial: sum_{j, i} | dS(j) - (x~_j . x^q_i - c~_j . c^q_i) |
     with x^ = x/||x|| per column, S_x(j) = HW - u_x . x^_j, u_x = sum_i x^_i,
     Sinv = 1/S, x~ = x^ * Sinv_x, dS(j) = Sinv_x(j) - Sinv_c(j)
  style partials: sim2(j, i) = s^_j . x^q_i
     m1sum = sum_i max_j sim2   (scalar, final per core)
     m2part(j) = max_{i in half} sim2  (host maxes the two i-halves)

Device layout: j on partitions (32 tiles of 128), i on free dim.
Single bf16 input [3C, HW]; single f32 output [128, 34]
(cols 0:32 m2part, col 32 content partial per j-partition, col 33 m1sum).
"""

import numpy as np

B, C, H, W = 4, 256, 64, 64
HW = H * W          # 4096
IQ = HW // 2        # 2048 query columns per core
NCORES = 8
NJT = HW // 128     # 32 j-tiles
NIT = 2             # i-tiles
IT = IQ // NIT      # 1024

_CACHED_NC = None


def _build(repeat=1):
    import concourse.bacc as bacc
    import concourse.tile as tile
    from concourse import mybir, bass_isa
    from concourse.alu_op_type import AluOpType
    from contextlib import ExitStack

    dt = mybir.dt
    AF = mybir.ActivationFunctionType
    AX = mybir.AxisListType

    nc = bacc.Bacc(None)

    xin = nc.dram_tensor("xin", [3 * C, HW], dt.bfloat16, kind="ExternalInput")
    o_all = nc.dram_tensor("oall", [128, NJT + 2], dt.float32,
                           kind="ExternalOutput")

    with tile.TileContext(nc) as tc, ExitStack() as top:
        pers = top.enter_context(tc.tile_pool(name="pers", bufs=1))
        for _rep in range(repeat):
            # ---------------- persistent tiles ----------------
            # content lhsT (j side): rows 0:256 = x~, 256:512 = -c~
            LC = [pers.tile([128, HW], dt.bfloat16, tag=f"lc{i}", name=f"LC{i}")
                  for i in range(4)]
            # style lhsT: s^
            LS = [pers.tile([128, HW], dt.bfloat16, tag=f"ls{i}", name=f"LS{i}")
                  for i in range(2)]
            # normalized columns (also the matmul rhs: cols 0:2048 = query)
            HX = [pers.tile([128, HW], dt.bfloat16, tag=f"hx{i}", name=f"HX{i}")
                  for i in range(2)]
            HC = [pers.tile([128, HW], dt.bfloat16, tag=f"hc{i}", name=f"HC{i}")
                  for i in range(2)]
            dSc = pers.tile([128, NJT], dt.float32, tag="dsc", name="dSc")
            csum_slots = pers.tile([128, NJT * NIT], dt.float32, tag="cslot",
                                   name="cslot")
            m2slots = pers.tile([128, NJT * NIT], dt.float32, tag="m2slot",
                                name="m2slot")
            m1acc = pers.tile([128, IQ], dt.float32, tag="m1acc", name="m1acc")
            ones_bf = pers.tile([128, 128], dt.bfloat16, tag="ones",
                                name="ones_bf")
            nc.vector.memset(ones_bf[:], 1.0)
            b4096 = pers.tile([128, 1], dt.float32, tag="b4096", name="b4096")
            nc.vector.memset(b4096[:], float(HW))

            sic = {}  # per-column 1/S in j-partition layout, for x and c

            # ---------------- preprocessing ----------------
            with tc.tile_pool(name="pre", bufs=1) as pre, \
                 tc.tile_pool(name="pps", bufs=2, space="PSUM") as pps:

                def process(name, row0, kind, dst):
                    """Load rows [row0, row0+C) of xin, normalize columns into
                    dst (2 tiles of [128, HW] bf16); kind: 'xc'|'s'."""
                    raw = []
                    for k in range(2):
                        t = pre.tile([128, HW], dt.bfloat16, tag=f"raw{k}",
                                     bufs=2, name=f"raw_{name}{k}")
                        r0 = row0 + k * 128
                        nc.sync.dma_start(t[:], xin[r0:r0 + 128, :])
                        raw.append(t)
                    # column rnorm = 1/||col||, broadcast on partitions
                    rn = pre.tile([128, HW], dt.float32, tag="rn", bufs=1,
                                  name=f"rn_{name}")
                    for h in range(2):
                        hs = slice(h * 2048, (h + 1) * 2048)
                        sq = []
                        for k in range(2):
                            q = pre.tile([128, 2048], dt.bfloat16,
                                         tag=f"sqh{k}", bufs=1,
                                         name=f"sq_{name}{h}{k}")
                            nc.gpsimd.tensor_mul(q[:], raw[k][:, hs],
                                                 raw[k][:, hs])
                            sq.append(q)
                        ns = pps.tile([128, 2048], dt.float32, tag="psbig",
                                      name=f"ns_{name}{h}")
                        for k in range(2):
                            for m in range(4):
                                nc.tensor.matmul(
                                    ns[:, m * 512:(m + 1) * 512],
                                    ones_bf[:],
                                    sq[k][:, m * 512:(m + 1) * 512],
                                    start=(k == 0), stop=(k == 1),
                                )
                        lt = pre.tile([128, 2048], dt.float32, tag="lntmp",
                                      bufs=1, name=f"lt_{name}{h}")
                        nc.scalar.activation(lt[:], ns[:], AF.Ln)
                        nc.scalar.activation(rn[:, hs], lt[:], AF.Exp,
                                             scale=-0.5)
                    # normalized columns (bf16) into dst
                    for k in range(2):
                        nc.vector.tensor_mul(dst[k][:], raw[k][:], rn[:])

                    if kind != "xc":
                        return

                    hat = dst
                    # u = sum_i hat_i  (ACT Copy + accum per half)
                    uh = [[pre.tile([128, 1], dt.float32, tag=f"u{k}h{h}",
                                    bufs=2, name=f"u_{name}{k}{h}")
                           for h in range(2)] for k in range(2)]
                    for k in range(2):
                        for h in range(2):
                            dump = pre.tile([128, 2048], dt.bfloat16,
                                            tag=f"sqh{k}", bufs=1,
                                            name=f"dump_{name}{k}{h}")
                            nc.scalar.activation(
                                dump[:], hat[k][:, h * 2048:(h + 1) * 2048],
                                AF.Copy, accum_out=uh[k][h][:])
                    u = [pre.tile([128, 1], dt.float32, tag=f"uu{k}", bufs=2,
                                  name=f"uu_{name}{k}") for k in range(2)]
                    u_bf = [pre.tile([128, 1], dt.bfloat16, tag=f"ub{k}",
                                     bufs=2, name=f"ub_{name}{k}")
                            for k in range(2)]
                    u_rep = [pre.tile([128, 128], dt.bfloat16, tag=f"ur{k}",
                                      bufs=2, name=f"ur_{name}{k}")
                             for k in range(2)]
                    for k in range(2):
                        nc.vector.tensor_add(u[k][:], uh[k][0][:], uh[k][1][:])
                        nc.vector.tensor_copy(u_bf[k][:], u[k][:])
                        nc.vector.tensor_scalar(u_rep[k][:], ones_bf[:],
                                                u[k][:], None,
                                                op0=AluOpType.mult)

                    # Sinv broadcast = exp(-ln(HW - u.hat_j))  [128, HW] f32
                    sinv = pre.tile([128, HW], dt.float32, tag="sinv", bufs=1,
                                    name=f"sinv_{name}")
                    for h in range(2):
                        hs = slice(h * 2048, (h + 1) * 2048)
                        su = pps.tile([128, 2048], dt.float32, tag="psbig",
                                      name=f"su_{name}{h}")
                        for k in range(2):
                            for m in range(4):
                                nc.tensor.matmul(
                                    su[:, m * 512:(m + 1) * 512],
                                    u_rep[k][:],
                                    hat[k][:, h * 2048 + m * 512:
                                           h * 2048 + (m + 1) * 512],
                                    start=(k == 0), stop=(k == 1),
                                )
                        lt = pre.tile([128, 2048], dt.float32, tag="lntmp",
                                      bufs=1, name=f"lts_{name}{h}")
                        nc.scalar.activation(lt[:], su[:], AF.Ln,
                                             bias=b4096[:], scale=-1.0)
                        nc.scalar.activation(sinv[:, hs], lt[:], AF.Exp,
                                             scale=-1.0)

                    # Su_cols [128, NJT] (j on partitions) -> 1/S exact
                    suc = pps.tile([128, NJT], dt.float32, tag="psbig",
                                   name=f"suc_{name}")
                    for jt in range(NJT):
                        for k in range(2):
                            nc.tensor.matmul(
                                suc[:, jt:jt + 1],
                                hat[k][:, jt * 128:(jt + 1) * 128],
                                u_bf[k][:],
                                start=(k == 0), stop=(k == 1),
                            )
                    scol = pre.tile([128, NJT], dt.float32, tag="scol", bufs=2,
                                    name=f"scol_{name}")
                    nc.vector.tensor_scalar(scol[:], suc[:], -1.0, float(HW),
                                            op0=AluOpType.mult,
                                            op1=AluOpType.add)
                    s_ic = pers.tile([128, NJT], dt.float32, tag=f"sic_{name}",
                                     name=f"sic_{name}")
                    nc.vector.reciprocal(s_ic[:], scol[:])
                    sic[name] = s_ic

                    # x~ = hat * Sinv into LC rows (negated for c so the
                    # content matmul sums x-part and c-part directly)
                    off = 0 if name == "x" else 2
                    for k in range(2):
                        if name == "x":
                            nc.vector.tensor_mul(LC[off + k][:], hat[k][:],
                                                 sinv[:])
                        else:
                            nc.vector.scalar_tensor_tensor(
                                LC[off + k][:], hat[k][:], -1.0, sinv[:],
                                op0=AluOpType.mult, op1=AluOpType.mult)

                process("x", 0, "xc", HX)
                process("c", C, "xc", HC)
                process("s", 2 * C, "s", LS)

                nc.vector.tensor_sub(dSc[:], sic["x"][:], sic["c"][:])

            # ---------------- main loop ----------------
            with tc.tile_pool(name="cps", bufs=2, space="PSUM") as cps, \
                 tc.tile_pool(name="sps", bufs=2, space="PSUM") as sps, \
                 tc.tile_pool(name="dmp", bufs=2) as dmp:
                RH = [HX[0], HX[1], HC[0], HC[1]]
                for jt in range(NJT):
                    js = slice(jt * 128, (jt + 1) * 128)
                    for it in range(NIT):
                        idx = jt * NIT + it
                        # content: G' = x~^T x^q - c~^T c^q   (K = 512)
                        psG = cps.tile([128, IT], dt.float32, tag="psG",
                                       name=f"psG{idx}")
                        for k in range(4):
                            for m in range(2):
                                fs = slice(it * IT + m * 512,
                                           it * IT + (m + 1) * 512)
                                nc.tensor.matmul(
                                    psG[:, m * 512:(m + 1) * 512],
                                    LC[k][:, js], RH[k][:, fs],
                                    start=(k == 0), stop=(k == 3),
                                )
                        dump = dmp.tile([128, IT], dt.bfloat16, tag="adump",
                                        name=f"adump{idx}")
                        nc.scalar.activation(
                            dump[:], psG[:], AF.Abs,
                            bias=dSc[:, jt:jt + 1], scale=-1.0,
                            accum_out=csum_slots[:, idx:idx + 1],
                        )
                        # style: sim2 = s^^T x^q   (K = 256)
                        psS = sps.tile([128, IT], dt.float32, tag="psS",
                                       name=f"psS{idx}")
                        for k in range(2):
                            for m in range(2):
                                fs = slice(it * IT + m * 512,
                                           it * IT + (m + 1) * 512)
                                nc.tensor.matmul(
                                    psS[:, m * 512:(m + 1) * 512],
                                    LS[k][:, js], HX[k][:, fs],
                                    start=(k == 0), stop=(k == 1),
                                )
                        nc.vector.reduce_max(m2slots[:, idx:idx + 1], psS[:],
                                             axis=AX.X)
                        sl = m1acc[:, it * IT:(it + 1) * IT]
                        if jt == 0:
                            nc.vector.tensor_copy(sl, psS[:])
                        else:
                            nc.vector.tensor_max(sl, sl, psS[:])

                # ---------------- finishers ----------------
                fin = dmp.tile([128, NJT + 2], dt.float32, tag="fin",
                               name="fin")
                m2v = m2slots[:].rearrange("p (j t) -> p j t", t=NIT)
                nc.vector.tensor_max(fin[:, 0:NJT], m2v[:, :, 0], m2v[:, :, 1])
                nc.vector.reduce_sum(fin[:, NJT:NJT + 1], csum_slots[:],
                                     axis=AX.X)
                m1r = dmp.tile([128, IQ], dt.float32, tag="m1r", name="m1r")
                nc.gpsimd.partition_all_reduce(
                    m1r[:], m1acc[:], channels=128,
                    reduce_op=bass_isa.ReduceOp.max)
                nc.vector.reduce_sum(fin[:, NJT + 1:NJT + 2], m1r[:],
                                     axis=AX.X)
                nc.sync.dma_start(o_all[:], fin[:])

    nc.finalize()
    return nc


def _get_nc():
    global _CACHED_NC
    if _CACHED_NC is None:
        import os
        _CACHED_NC = _build(repeat=int(os.environ.get("KREPEAT", "1")))
    return _CACHED_NC


_RUNNER = None


def _get_runner():
    """Compile the 8-core PJRT executable once; returns run(in_maps)->results.

    Mirrors concourse.bass2jax.run_bass_via_pjrt but AOT-compiles with
    bass_effect suppressed (fast C++ dispatch) and caches the executable
    so repeated kernel() calls only pay device execution.
    """
    global _RUNNER
    if _RUNNER is not None:
        return _RUNNER
    import jax
    import numpy as _np
    from jax.sharding import Mesh, PartitionSpec
    from jax.experimental.shard_map import shard_map
    from concourse import mybir, bass2jax
    from concourse.bass2jax import (_bass_exec_p, partition_id_tensor,
                                    fast_dispatch_compile)

    bass2jax.install_neuronx_cc_hook()
    nc = _get_nc()
    partition_name = (nc.partition_id_tensor.name
                      if nc.partition_id_tensor else None)

    in_names, out_names, out_avals, zero_outs = [], [], [], []
    in_shapes = []
    for alloc in nc.m.functions[0].allocations:
        if not isinstance(alloc, mybir.MemoryLocationSet):
            continue
        name = alloc.memorylocations[0].name
        if alloc.kind == "ExternalInput":
            if name != partition_name:
                in_names.append(name)
                in_shapes.append((tuple(alloc.tensor_shape),
                                  mybir.dt.np(alloc.dtype)))
        elif alloc.kind == "ExternalOutput":
            out_names.append(name)
            shape = tuple(alloc.tensor_shape)
            dtype = mybir.dt.np(alloc.dtype)
            out_avals.append(jax.core.ShapedArray(shape, dtype))
            zero_outs.append(_np.zeros((NCORES * shape[0], *shape[1:]), dtype))
    n_params = len(in_names)
    n_outs = len(out_avals)
    all_names = list(in_names) + list(out_names)
    if partition_name is not None:
        all_names.append(partition_name)

    def _body(*args):
        operands = list(args)
        if partition_name is not None:
            operands.append(partition_id_tensor())
        return tuple(_bass_exec_p.bind(
            *operands,
            out_avals=tuple(out_avals),
            in_names=tuple(all_names),
            out_names=tuple(out_names),
            lowering_input_output_aliases=(),
            sim_require_finite=True,
            sim_require_nnan=True,
            nc=nc,
        ))

    devices = jax.devices()[:NCORES]
    mesh = Mesh(_np.asarray(devices), ("core",))
    from jax.sharding import NamedSharding
    sh = NamedSharding(mesh, PartitionSpec("core"))

    # AOT-compile with bass_effect suppressed: the default effectful path
    # forces slow Python dispatch with effect tokens on every call; the
    # fast path dispatches through C++ (see bass2jax.fast_dispatch_compile).
    arg_structs = (
        [jax.ShapeDtypeStruct((NCORES * s[0], *s[1:]), dt, sharding=sh)
         for s, dt in in_shapes]
        + [jax.ShapeDtypeStruct((NCORES * a.shape[0], *a.shape[1:]), a.dtype,
                                sharding=sh) for a in out_avals]
    )

    def _compile():
        return jax.jit(
            shard_map(_body, mesh=mesh,
                      in_specs=(PartitionSpec("core"),) * (n_params + n_outs),
                      out_specs=(PartitionSpec("core"),) * n_outs,
                      check_rep=False),
            keep_unused=True,
        ).lower(*arg_structs).compile()

    sharded = fast_dispatch_compile(_compile)
    zero_dev = [jax.device_put(a, sh) for a in zero_outs]

    def prepare(in_maps):
        """Stage concatenated inputs onto the devices once (for timing)."""
        concat_in = [
            _np.concatenate([in_maps[c][nm] for c in range(NCORES)], axis=0)
            for nm in in_names
        ]
        return [jax.device_put(a, sh) for a in concat_in]

    def exec_prepared(staged):
        out_arrs = sharded(*staged, *zero_dev)
        jax.block_until_ready(out_arrs)
        return out_arrs

    def exec_async(staged):
        """Dispatch one execution without blocking (pipelined timing)."""
        return sharded(*staged, *zero_dev)

    def run(in_maps):
        concat_in = [
            jax.device_put(
                _np.concatenate([in_maps[c][nm] for c in range(NCORES)],
                                axis=0), sh)
            for nm in in_names
        ]
        out_arrs = sharded(*concat_in, *zero_dev)
        jax.block_until_ready(out_arrs)
        return [
            {nm: _np.asarray(out_arrs[i]).reshape(NCORES, *out_avals[i].shape)[c]
             for i, nm in enumerate(out_names)}
            for c in range(NCORES)
        ]

    run.prepare = prepare
    run.exec_prepared = exec_prepared
    run.exec_async = exec_async
    _RUNNER = run
    return run


def _make_in_maps(x_feat, c_feat, s_feat):
    import ml_dtypes
    x = np.asarray(x_feat, dtype=np.float32).reshape(B, C, HW)
    c = np.asarray(c_feat, dtype=np.float32).reshape(B, C, HW)
    s = np.asarray(s_feat, dtype=np.float32).reshape(B, C, HW)
    in_maps = []
    for k in range(NCORES):
        b, ih = k // 2, k % 2
        parts = []
        for a in (x, c, s):
            ab = a[b]
            if ih:
                ab = np.roll(ab, -IQ, axis=1)
            parts.append(ab)
        xin = np.concatenate(parts, axis=0).astype(ml_dtypes.bfloat16)
        in_maps.append({"xin": np.ascontiguousarray(xin)})
    return in_maps


def kernel(x_feat, c_feat, s_feat):
    outs = _get_runner()(_make_in_maps(x_feat, c_feat, s_feat))

    total = sum(float(r["oall"][:, NJT].sum()) for r in outs)
    content = total / (B * HW)

    m1total = sum(float(r["oall"][0, NJT + 1]) for r in outs)
    m1mean = 1.0 - m1total / (B * HW)
    m2mean = 0.0
    for b_ in range(B):
        flats = []
        for ih in range(2):
            dev = outs[2 * b_ + ih]["oall"][:, :NJT]  # [128 p, 32 jt]
            flat = dev.T.ravel()  # index j_dev = jt*128 + p
            flats.append(np.roll(flat, IQ * ih))
        mx = np.maximum(flats[0], flats[1])
        m2mean += float((1.0 - mx).mean())
    m2mean /= B
    style = max(m1mean, m2mean)

    return (np.float32(content), np.float32(style))


# revision 12
# speedup vs baseline: 116.0617x; 1.0111x over previous
"""ContentStyleReltLoss kernel for 8 Trainium2 NeuronCores.

Sharding: core k handles (batch b = k//2, query-half ih = k%2).
Host rolls the HW columns of x/c/s by -ih*2048 so every core's 2048
"query" columns sit at device columns 0:2048 — all cores run identical
code. Each core computes, for its 2048 query columns i against ALL
4096 "key" columns j:

  content partial: sum_{j, i} | dS(j) - (x~_j . x^q_i - c~_j . c^q_i) |
     with x^ = x/||x|| per column, S_x(j) = HW - u_x . x^_j, u_x = sum_i x^_i,
     Sinv = 1/S, x~ = x^ * Sinv_x, dS(j) = Sinv_x(j) - Sinv_c(j)
  style partials: sim2(j, i) = s^_j . x^q_i
     m1sum = sum_i max_j sim2   (scalar, final per core)
     m2part(j) = max_{i in half} sim2  (host maxes the two i-halves)

Device layout: j on partitions (32 tiles of 128), i on free dim.
Single bf16 input [3C, HW]; single f32 output [128, 34]
(cols 0:32 m2part, col 32 content partial per j-partition, col 33 m1sum).
"""

import numpy as np

B, C, H, W = 4, 256, 64, 64
HW = H * W          # 4096
IQ = HW // 2        # 2048 query columns per core
NCORES = 8
NJT = HW // 128     # 32 j-tiles
NIT = 2             # i-tiles
IT = IQ // NIT      # 1024

_CACHED_NC = None


def _build(repeat=1):
    import concourse.bacc as bacc
    import concourse.tile as tile
    from concourse import mybir, bass_isa
    from concourse.alu_op_type import AluOpType
    from contextlib import ExitStack

    dt = mybir.dt
    AF = mybir.ActivationFunctionType
    AX = mybir.AxisListType

    nc = bacc.Bacc(None)

    xin = nc.dram_tensor("xin", [3 * C, HW], dt.bfloat16, kind="ExternalInput")
    o_all = nc.dram_tensor("oall", [128, NJT + 2], dt.float32,
                           kind="ExternalOutput")

    F8 = dt.float8e4
    PM = mybir.MatmulPerfMode.DoubleRow

    with tile.TileContext(nc) as tc, ExitStack() as top:
        pers = top.enter_context(tc.tile_pool(name="pers", bufs=1))
        for _rep in range(repeat):
            # ---------------- persistent tiles ----------------
            # fp8 DoubleRow operands: dim1 = channel-half (K subtile).
            # content lhsT: x~ * 2^16  and  -c~ * 2^16
            LCX8 = pers.tile([128, 2, HW], F8, tag="lcx8", name="LCX8")
            LCC8 = pers.tile([128, 2, HW], F8, tag="lcc8", name="LCC8")
            # style lhsT: s^ * 2^4
            LS8 = pers.tile([128, 2, HW], F8, tag="ls8", name="LS8")
            # rhs (query cols): x^q * 2^4, c^q * 2^4
            RX8 = pers.tile([128, 2, IQ], F8, tag="rx8", name="RX8")
            RC8 = pers.tile([128, 2, IQ], F8, tag="rc8", name="RC8")
            # normalized columns bf16 (preprocessing operands)
            HX = [pers.tile([128, HW], dt.bfloat16, tag=f"hx{i}", name=f"HX{i}")
                  for i in range(2)]
            HC = [pers.tile([128, HW], dt.bfloat16, tag=f"hc{i}", name=f"HC{i}")
                  for i in range(2)]
            dSc = pers.tile([128, NJT], dt.float32, tag="dsc", name="dSc")
            csum_slots = pers.tile([128, NJT * NIT], dt.float32, tag="cslot",
                                   name="cslot")
            m2slots = pers.tile([128, NJT * NIT], dt.float32, tag="m2slot",
                                name="m2slot")
            m1acc = pers.tile([128, IQ], dt.float32, tag="m1acc", name="m1acc")
            ones_bf = pers.tile([128, 128], dt.bfloat16, tag="ones",
                                name="ones_bf")
            nc.vector.memset(ones_bf[:], 1.0)
            b4096 = pers.tile([128, 1], dt.float32, tag="b4096", name="b4096")
            nc.vector.memset(b4096[:], float(HW))

            sic = {}  # per-column 1/S in j-partition layout, for x and c

            # ---------------- preprocessing ----------------
            with tc.tile_pool(name="pre", bufs=1) as pre, \
                 tc.tile_pool(name="pps", bufs=2, space="PSUM") as pps:

                def process(name, row0, kind, dst):
                    """Load rows [row0, row0+C) of xin, normalize columns.
                    kind 'xc': dst = 2 bf16 [128, HW] hat tiles;
                    kind 's': write s^ * 16 into LS8 (fp8) directly."""
                    raw = []
                    for k in range(2):
                        t = pre.tile([128, HW], dt.bfloat16, tag=f"raw{k}",
                                     bufs=2, name=f"raw_{name}{k}")
                        r0 = row0 + k * 128
                        nc.sync.dma_start(t[:], xin[r0:r0 + 128, :])
                        raw.append(t)
                    # column rnorm = 1/||col||, broadcast on partitions
                    rn = pre.tile([128, HW], dt.float32, tag="rn", bufs=1,
                                  name=f"rn_{name}")
                    for h in range(2):
                        hs = slice(h * 2048, (h + 1) * 2048)
                        sq = []
                        for k in range(2):
                            q = pre.tile([128, 2048], dt.bfloat16,
                                         tag=f"sqh{k}", bufs=1,
                                         name=f"sq_{name}{h}{k}")
                            nc.gpsimd.tensor_mul(q[:], raw[k][:, hs],
                                                 raw[k][:, hs])
                            sq.append(q)
                        ns = pps.tile([128, 2048], dt.float32, tag="psbig",
                                      name=f"ns_{name}{h}")
                        for k in range(2):
                            for m in range(4):
                                nc.tensor.matmul(
                                    ns[:, m * 512:(m + 1) * 512],
                                    ones_bf[:],
                                    sq[k][:, m * 512:(m + 1) * 512],
                                    start=(k == 0), stop=(k == 1),
                                )
                        lt = pre.tile([128, 2048], dt.float32, tag="lntmp",
                                      bufs=1, name=f"lt_{name}{h}")
                        nc.scalar.activation(lt[:], ns[:], AF.Ln)
                        nc.scalar.activation(rn[:, hs], lt[:], AF.Exp,
                                             scale=-0.5)
                    if kind == "s":
                        for k in range(2):
                            nc.vector.scalar_tensor_tensor(
                                LS8[:, k, :], raw[k][:], 16.0, rn[:],
                                op0=AluOpType.mult, op1=AluOpType.mult)
                        return
                    # normalized columns (bf16) into dst
                    for k in range(2):
                        nc.vector.tensor_mul(dst[k][:], raw[k][:], rn[:])

                    hat = dst
                    # u = sum_i hat_i  (ACT Copy + accum per half)
                    uh = [[pre.tile([128, 1], dt.float32, tag=f"u{k}h{h}",
                                    bufs=2, name=f"u_{name}{k}{h}")
                           for h in range(2)] for k in range(2)]
                    for k in range(2):
                        for h in range(2):
                            dump = pre.tile([128, 2048], dt.bfloat16,
                                            tag=f"sqh{k}", bufs=1,
                                            name=f"dump_{name}{k}{h}")
                            nc.scalar.activation(
                                dump[:], hat[k][:, h * 2048:(h + 1) * 2048],
                                AF.Copy, accum_out=uh[k][h][:])
                    u = [pre.tile([128, 1], dt.float32, tag=f"uu{k}", bufs=2,
                                  name=f"uu_{name}{k}") for k in range(2)]
                    u_bf = [pre.tile([128, 1], dt.bfloat16, tag=f"ub{k}",
                                     bufs=2, name=f"ub_{name}{k}")
                            for k in range(2)]
                    u_rep = [pre.tile([128, 128], dt.bfloat16, tag=f"ur{k}",
                                      bufs=2, name=f"ur_{name}{k}")
                             for k in range(2)]
                    for k in range(2):
                        nc.vector.tensor_add(u[k][:], uh[k][0][:], uh[k][1][:])
                        nc.vector.tensor_copy(u_bf[k][:], u[k][:])
                        nc.vector.tensor_scalar(u_rep[k][:], ones_bf[:],
                                                u[k][:], None,
                                                op0=AluOpType.mult)

                    # Sinv broadcast = exp(-ln(HW - u.hat_j))  [128, HW] f32
                    sinv = pre.tile([128, HW], dt.float32, tag="sinv", bufs=1,
                                    name=f"sinv_{name}")
                    for h in range(2):
                        hs = slice(h * 2048, (h + 1) * 2048)
                        su = pps.tile([128, 2048], dt.float32, tag="psbig",
                                      name=f"su_{name}{h}")
                        for k in range(2):
                            for m in range(4):
                                nc.tensor.matmul(
                                    su[:, m * 512:(m + 1) * 512],
                                    u_rep[k][:],
                                    hat[k][:, h * 2048 + m * 512:
                                           h * 2048 + (m + 1) * 512],
                                    start=(k == 0), stop=(k == 1),
                                )
                        lt = pre.tile([128, 2048], dt.float32, tag="lntmp",
                                      bufs=1, name=f"lts_{name}{h}")
                        nc.scalar.activation(lt[:], su[:], AF.Ln,
                                             bias=b4096[:], scale=-1.0)
                        nc.scalar.activation(sinv[:, hs], lt[:], AF.Exp,
                                             scale=-1.0)

                    # Su_cols [128, NJT] (j on partitions) -> 1/S exact
                    suc = pps.tile([128, NJT], dt.float32, tag="psbig",
                                   name=f"suc_{name}")
                    for jt in range(NJT):
                        for k in range(2):
                            nc.tensor.matmul(
                                suc[:, jt:jt + 1],
                                hat[k][:, jt * 128:(jt + 1) * 128],
                                u_bf[k][:],
                                start=(k == 0), stop=(k == 1),
                            )
                    scol = pre.tile([128, NJT], dt.float32, tag="scol", bufs=2,
                                    name=f"scol_{name}")
                    nc.vector.tensor_scalar(scol[:], suc[:], -1.0, float(HW),
                                            op0=AluOpType.mult,
                                            op1=AluOpType.add)
                    s_ic = pers.tile([128, NJT], dt.float32, tag=f"sic_{name}",
                                     name=f"sic_{name}")
                    nc.vector.reciprocal(s_ic[:], scol[:])
                    sic[name] = s_ic

                    # content lhsT = hat * Sinv * 2^16 in fp8 (negated for c
                    # so the content matmul sums x-part and c-part directly);
                    # rhs = query cols * 2^4 in fp8.
                    lc8 = LCX8 if name == "x" else LCC8
                    sgn = 65536.0 if name == "x" else -65536.0
                    r8 = RX8 if name == "x" else RC8
                    for k in range(2):
                        nc.vector.scalar_tensor_tensor(
                            lc8[:, k, :], hat[k][:], sgn, sinv[:],
                            op0=AluOpType.mult, op1=AluOpType.mult)
                        nc.scalar.activation(r8[:, k, :], hat[k][:, 0:IQ],
                                             AF.Copy, scale=16.0)

                process("x", 0, "xc", HX)
                process("c", C, "xc", HC)
                process("s", 2 * C, "s", None)

                nc.vector.tensor_sub(dSc[:], sic["x"][:], sic["c"][:])

            # ---------------- main loop ----------------
            with tc.tile_pool(name="cps", bufs=2, space="PSUM") as cps, \
                 tc.tile_pool(name="sps", bufs=2, space="PSUM") as sps, \
                 tc.tile_pool(name="dmp", bufs=2) as dmp:
                for jt in range(NJT):
                    js = slice(jt * 128, (jt + 1) * 128)
                    for it in range(NIT):
                        idx = jt * NIT + it
                        # content: psG = 2^20 (x~^T x^q - c~^T c^q), fp8
                        # DoubleRow (K = 256 per instruction)
                        psG = cps.tile([128, IT], dt.float32, tag="psG",
                                       name=f"psG{idx}")
                        for m in range(2):
                            fs = slice(it * IT + m * 512,
                                       it * IT + (m + 1) * 512)
                            nc.tensor.matmul(
                                psG[:, m * 512:(m + 1) * 512],
                                LCX8[:, :, js], RX8[:, :, fs],
                                start=True, stop=False, perf_mode=PM,
                            )
                            nc.tensor.matmul(
                                psG[:, m * 512:(m + 1) * 512],
                                LCC8[:, :, js], RC8[:, :, fs],
                                start=False, stop=True, perf_mode=PM,
                            )
                        dump = dmp.tile([128, IT], dt.bfloat16, tag="adump",
                                        name=f"adump{idx}")
                        nc.scalar.activation(
                            dump[:], psG[:], AF.Abs,
                            bias=dSc[:, jt:jt + 1], scale=-(2.0 ** -20),
                            accum_out=csum_slots[:, idx:idx + 1],
                        )
                        # style: psS = 2^8 s^^T x^q, fp8 DoubleRow (K = 256)
                        psS = sps.tile([128, IT], dt.float32, tag="psS",
                                       name=f"psS{idx}")
                        for m in range(2):
                            fs = slice(it * IT + m * 512,
                                       it * IT + (m + 1) * 512)
                            nc.tensor.matmul(
                                psS[:, m * 512:(m + 1) * 512],
                                LS8[:, :, js], RX8[:, :, fs],
                                start=True, stop=True, perf_mode=PM,
                            )
                        nc.vector.reduce_max(m2slots[:, idx:idx + 1], psS[:],
                                             axis=AX.X)
                        sl = m1acc[:, it * IT:(it + 1) * IT]
                        if jt == 0:
                            nc.vector.tensor_copy(sl, psS[:])
                        else:
                            nc.vector.tensor_max(sl, sl, psS[:])

                # ---------------- finishers ----------------
                fin = dmp.tile([128, NJT + 2], dt.float32, tag="fin",
                               name="fin")
                m2v = m2slots[:].rearrange("p (j t) -> p j t", t=NIT)
                nc.vector.tensor_max(fin[:, 0:NJT], m2v[:, :, 0], m2v[:, :, 1])
                nc.vector.reduce_sum(fin[:, NJT:NJT + 1], csum_slots[:],
                                     axis=AX.X)
                m1r = dmp.tile([128, IQ], dt.float32, tag="m1r", name="m1r")
                nc.gpsimd.partition_all_reduce(
                    m1r[:], m1acc[:], channels=128,
                    reduce_op=bass_isa.ReduceOp.max)
                nc.vector.reduce_sum(fin[:, NJT + 1:NJT + 2], m1r[:],
                                     axis=AX.X)
                nc.sync.dma_start(o_all[:], fin[:])

    nc.finalize()
    return nc


def _get_nc():
    global _CACHED_NC
    if _CACHED_NC is None:
        import os
        _CACHED_NC = _build(repeat=int(os.environ.get("KREPEAT", "1")))
    return _CACHED_NC


_RUNNER = None


def _get_runner():
    """Compile the 8-core PJRT executable once; returns run(in_maps)->results.

    Mirrors concourse.bass2jax.run_bass_via_pjrt but AOT-compiles with
    bass_effect suppressed (fast C++ dispatch) and caches the executable
    so repeated kernel() calls only pay device execution.
    """
    global _RUNNER
    if _RUNNER is not None:
        return _RUNNER
    import jax
    import numpy as _np
    from jax.sharding import Mesh, PartitionSpec
    from jax.experimental.shard_map import shard_map
    from concourse import mybir, bass2jax
    from concourse.bass2jax import (_bass_exec_p, partition_id_tensor,
                                    fast_dispatch_compile)

    bass2jax.install_neuronx_cc_hook()
    nc = _get_nc()
    partition_name = (nc.partition_id_tensor.name
                      if nc.partition_id_tensor else None)

    in_names, out_names, out_avals, zero_outs = [], [], [], []
    in_shapes = []
    for alloc in nc.m.functions[0].allocations:
        if not isinstance(alloc, mybir.MemoryLocationSet):
            continue
        name = alloc.memorylocations[0].name
        if alloc.kind == "ExternalInput":
            if name != partition_name:
                in_names.append(name)
                in_shapes.append((tuple(alloc.tensor_shape),
                                  mybir.dt.np(alloc.dtype)))
        elif alloc.kind == "ExternalOutput":
            out_names.append(name)
            shape = tuple(alloc.tensor_shape)
            dtype = mybir.dt.np(alloc.dtype)
            out_avals.append(jax.core.ShapedArray(shape, dtype))
            zero_outs.append(_np.zeros((NCORES * shape[0], *shape[1:]), dtype))
    n_params = len(in_names)
    n_outs = len(out_avals)
    all_names = list(in_names) + list(out_names)
    if partition_name is not None:
        all_names.append(partition_name)

    def _body(*args):
        operands = list(args)
        if partition_name is not None:
            operands.append(partition_id_tensor())
        return tuple(_bass_exec_p.bind(
            *operands,
            out_avals=tuple(out_avals),
            in_names=tuple(all_names),
            out_names=tuple(out_names),
            lowering_input_output_aliases=(),
            sim_require_finite=True,
            sim_require_nnan=True,
            nc=nc,
        ))

    devices = jax.devices()[:NCORES]
    mesh = Mesh(_np.asarray(devices), ("core",))
    from jax.sharding import NamedSharding
    sh = NamedSharding(mesh, PartitionSpec("core"))

    # AOT-compile with bass_effect suppressed: the default effectful path
    # forces slow Python dispatch with effect tokens on every call; the
    # fast path dispatches through C++ (see bass2jax.fast_dispatch_compile).
    arg_structs = (
        [jax.ShapeDtypeStruct((NCORES * s[0], *s[1:]), dt, sharding=sh)
         for s, dt in in_shapes]
        + [jax.ShapeDtypeStruct((NCORES * a.shape[0], *a.shape[1:]), a.dtype,
                                sharding=sh) for a in out_avals]
    )

    def _compile():
        return jax.jit(
            shard_map(_body, mesh=mesh,
                      in_specs=(PartitionSpec("core"),) * (n_params + n_outs),
                      out_specs=(PartitionSpec("core"),) * n_outs,
                      check_rep=False),
            keep_unused=True,
        ).lower(*arg_structs).compile()

    sharded = fast_dispatch_compile(_compile)
    zero_dev = [jax.device_put(a, sh) for a in zero_outs]

    def prepare(in_maps):
        """Stage concatenated inputs onto the devices once (for timing)."""
        concat_in = [
            _np.concatenate([in_maps[c][nm] for c in range(NCORES)], axis=0)
            for nm in in_names
        ]
        return [jax.device_put(a, sh) for a in concat_in]

    def exec_prepared(staged):
        out_arrs = sharded(*staged, *zero_dev)
        jax.block_until_ready(out_arrs)
        return out_arrs

    def exec_async(staged):
        """Dispatch one execution without blocking (pipelined timing)."""
        return sharded(*staged, *zero_dev)

    def run(in_maps):
        concat_in = [
            jax.device_put(
                _np.concatenate([in_maps[c][nm] for c in range(NCORES)],
                                axis=0), sh)
            for nm in in_names
        ]
        out_arrs = sharded(*concat_in, *zero_dev)
        jax.block_until_ready(out_arrs)
        return [
            {nm: _np.asarray(out_arrs[i]).reshape(NCORES, *out_avals[i].shape)[c]
             for i, nm in enumerate(out_names)}
            for c in range(NCORES)
        ]

    run.prepare = prepare
    run.exec_prepared = exec_prepared
    run.exec_async = exec_async
    _RUNNER = run
    return run


def _make_in_maps(x_feat, c_feat, s_feat):
    import ml_dtypes
    x = np.asarray(x_feat, dtype=np.float32).reshape(B, C, HW)
    c = np.asarray(c_feat, dtype=np.float32).reshape(B, C, HW)
    s = np.asarray(s_feat, dtype=np.float32).reshape(B, C, HW)
    in_maps = []
    for k in range(NCORES):
        b, ih = k // 2, k % 2
        parts = []
        for a in (x, c, s):
            ab = a[b]
            if ih:
                ab = np.roll(ab, -IQ, axis=1)
            parts.append(ab)
        xin = np.concatenate(parts, axis=0).astype(ml_dtypes.bfloat16)
        in_maps.append({"xin": np.ascontiguousarray(xin)})
    return in_maps


def kernel(x_feat, c_feat, s_feat):
    outs = _get_runner()(_make_in_maps(x_feat, c_feat, s_feat))

    total = sum(float(r["oall"][:, NJT].sum()) for r in outs)
    content = total / (B * HW)

    # style partials carry the fp8 scale 2^4 * 2^4 = 256
    m1total = sum(float(r["oall"][0, NJT + 1]) for r in outs) / 256.0
    m1mean = 1.0 - m1total / (B * HW)
    m2mean = 0.0
    for b_ in range(B):
        flats = []
        for ih in range(2):
            dev = outs[2 * b_ + ih]["oall"][:, :NJT]  # [128 p, 32 jt]
            flat = dev.T.ravel() / 256.0  # index j_dev = jt*128 + p
            flats.append(np.roll(flat, IQ * ih))
        mx = np.maximum(flats[0], flats[1])
        m2mean += float((1.0 - mx).mean())
    m2mean /= B
    style = max(m1mean, m2mean)

    return (np.float32(content), np.float32(style))


# revision 17
# speedup vs baseline: 123.3786x; 1.0630x over previous
"""ContentStyleReltLoss kernel for 8 Trainium2 NeuronCores.

Sharding: core k handles (batch b = k//2, query-half ih = k%2).
Host rolls the HW columns of x/c/s by -ih*2048 so every core's 2048
"query" columns sit at device columns 0:2048 — all cores run identical
code. Each core computes, for its 2048 query columns i against ALL
4096 "key" columns j:

  content partial: sum_{j, i} | dS(j) - (x~_j . x^q_i - c~_j . c^q_i) |
     with x^ = x/||x|| per column, S_x(j) = HW - u_x . x^_j, u_x = sum_i x^_i,
     Sinv = 1/S, x~ = x^ * Sinv_x, dS(j) = Sinv_x(j) - Sinv_c(j)
  style partials: sim2(j, i) = s^_j . x^q_i
     m1sum = sum_i max_j sim2   (scalar, final per core)
     m2part(j) = max_{i in half} sim2  (host maxes the two i-halves)

Device layout: j on partitions (32 tiles of 128), i on free dim.
Single bf16 input [3C, HW]; single f32 output [128, 34]
(cols 0:32 m2part, col 32 content partial per j-partition, col 33 m1sum).
"""

import numpy as np

B, C, H, W = 4, 256, 64, 64
HW = H * W          # 4096
IQ = HW // 2        # 2048 query columns per core
NCORES = 8
NJT = HW // 128     # 32 j-tiles
NIT = 2             # i-tiles
IT = IQ // NIT      # 1024

_CACHED_NC = None


def _build(repeat=1):
    import concourse.bacc as bacc
    import concourse.tile as tile
    from concourse import mybir, bass_isa
    from concourse.alu_op_type import AluOpType
    from contextlib import ExitStack

    dt = mybir.dt
    AF = mybir.ActivationFunctionType
    AX = mybir.AxisListType

    nc = bacc.Bacc(None)

    xin = nc.dram_tensor("xin", [3 * C, HW], dt.bfloat16, kind="ExternalInput")
    # cols 0:NJT m2part, col NJT content partial, cols NJT+1: raw m1acc
    # (host reduces max over partitions / sums)
    o_all = nc.dram_tensor("oall", [128, NJT + 1 + IQ], dt.float32,
                           kind="ExternalOutput")

    F8 = dt.float8e4
    PM = mybir.MatmulPerfMode.DoubleRow

    with tile.TileContext(nc) as tc, ExitStack() as top:
        pers = top.enter_context(tc.tile_pool(name="pers", bufs=1))
        for _rep in range(repeat):
            # ---------------- persistent tiles ----------------
            # fp8 DoubleRow operands: dim1 = channel-half (K subtile).
            # content lhsT: x~ * 2^16  and  -c~ * 2^16
            LCX8 = pers.tile([128, 2, HW], F8, tag="lcx8", name="LCX8")
            LCC8 = pers.tile([128, 2, HW], F8, tag="lcc8", name="LCC8")
            # style lhsT: s^ * 2^4
            LS8 = pers.tile([128, 2, HW], F8, tag="ls8", name="LS8")
            # rhs (query cols): x^q * 2^4, c^q * 2^4
            RX8 = pers.tile([128, 2, IQ], F8, tag="rx8", name="RX8")
            RC8 = pers.tile([128, 2, IQ], F8, tag="rc8", name="RC8")
            # normalized columns bf16 (preprocessing operands)
            HX = [pers.tile([128, HW], dt.bfloat16, tag=f"hx{i}", name=f"HX{i}")
                  for i in range(2)]
            HC = [pers.tile([128, HW], dt.bfloat16, tag=f"hc{i}", name=f"HC{i}")
                  for i in range(2)]
            dSc = pers.tile([128, NJT], dt.float32, tag="dsc", name="dSc")
            csum_slots = pers.tile([128, NJT * NIT], dt.float32, tag="cslot",
                                   name="cslot")
            m2slots = pers.tile([128, NJT * NIT], dt.float32, tag="m2slot",
                                name="m2slot")
            m1acc = pers.tile([128, IQ], dt.float32, tag="m1acc", name="m1acc")
            ones_bf = pers.tile([128, 128], dt.bfloat16, tag="ones",
                                name="ones_bf")
            nc.vector.memset(ones_bf[:], 1.0)
            b4096 = pers.tile([128, 1], dt.float32, tag="b4096", name="b4096")
            nc.vector.memset(b4096[:], float(HW))

            sic = {}  # per-column 1/S in j-partition layout, for x and c

            # ---------------- preprocessing ----------------
            with tc.tile_pool(name="pre", bufs=1) as pre, \
                 tc.tile_pool(name="pps", bufs=2, space="PSUM") as pps:

                def process(name, row0, kind, dst):
                    """Load rows [row0, row0+C) of xin, normalize columns.
                    kind 'xc': dst = 2 bf16 [128, HW] hat tiles;
                    kind 's': write s^ * 16 into LS8 (fp8) directly."""
                    raw = []
                    for k in range(2):
                        t = pre.tile([128, HW], dt.bfloat16, tag=f"raw{k}",
                                     bufs=2, name=f"raw_{name}{k}")
                        r0 = row0 + k * 128
                        nc.sync.dma_start(t[:], xin[r0:r0 + 128, :])
                        raw.append(t)
                    # column rnorm = 1/||col||, broadcast on partitions
                    rn = pre.tile([128, HW], dt.float32, tag="rn", bufs=1,
                                  name=f"rn_{name}")
                    for h in range(2):
                        hs = slice(h * 2048, (h + 1) * 2048)
                        sq = []
                        for k in range(2):
                            q = pre.tile([128, 2048], dt.bfloat16,
                                         tag=f"sqh{k}", bufs=1,
                                         name=f"sq_{name}{h}{k}")
                            nc.gpsimd.tensor_mul(q[:], raw[k][:, hs],
                                                 raw[k][:, hs])
                            sq.append(q)
                        ns = pps.tile([128, 2048], dt.float32, tag="psbig",
                                      name=f"ns_{name}{h}")
                        for k in range(2):
                            for m in range(4):
                                nc.tensor.matmul(
                                    ns[:, m * 512:(m + 1) * 512],
                                    ones_bf[:],
                                    sq[k][:, m * 512:(m + 1) * 512],
                                    start=(k == 0), stop=(k == 1),
                                )
                        lt = pre.tile([128, 2048], dt.float32, tag="lntmp",
                                      bufs=1, name=f"lt_{name}{h}")
                        nc.scalar.activation(lt[:], ns[:], AF.Ln)
                        nc.scalar.activation(rn[:, hs], lt[:], AF.Exp,
                                             scale=-0.5)
                    if kind == "s":
                        for k in range(2):
                            nc.vector.scalar_tensor_tensor(
                                LS8[:, k, :], raw[k][:], 16.0, rn[:],
                                op0=AluOpType.mult, op1=AluOpType.mult)
                        return
                    # normalized columns (bf16) into dst
                    for k in range(2):
                        nc.vector.tensor_mul(dst[k][:], raw[k][:], rn[:])

                    hat = dst
                    # fp8 rhs (query cols * 2^4) as soon as hat is ready
                    r8 = RX8 if name == "x" else RC8
                    for k in range(2):
                        nc.scalar.activation(r8[:, k, :], hat[k][:, 0:IQ],
                                             AF.Copy, scale=16.0)
                    # u = sum_i hat_i
                    u = [pre.tile([128, 1], dt.float32, tag=f"uu{k}", bufs=2,
                                  name=f"uu_{name}{k}") for k in range(2)]
                    u_bf = [pre.tile([128, 1], dt.bfloat16, tag=f"ub{k}",
                                     bufs=2, name=f"ub_{name}{k}")
                            for k in range(2)]
                    u_rep = [pre.tile([128, 128], dt.bfloat16, tag=f"ur{k}",
                                      bufs=2, name=f"ur_{name}{k}")
                             for k in range(2)]
                    for k in range(2):
                        nc.vector.reduce_sum(u[k][:], hat[k][:], axis=AX.X)
                        nc.vector.tensor_copy(u_bf[k][:], u[k][:])
                        nc.vector.tensor_scalar(u_rep[k][:], ones_bf[:],
                                                u[k][:], None,
                                                op0=AluOpType.mult)

                    # Sinv broadcast = exp(-ln(HW - u.hat_j))  [128, HW] f32
                    sinv = pre.tile([128, HW], dt.float32, tag="sinv", bufs=1,
                                    name=f"sinv_{name}")
                    for h in range(2):
                        hs = slice(h * 2048, (h + 1) * 2048)
                        su = pps.tile([128, 2048], dt.float32, tag="psbig",
                                      name=f"su_{name}{h}")
                        for k in range(2):
                            for m in range(4):
                                nc.tensor.matmul(
                                    su[:, m * 512:(m + 1) * 512],
                                    u_rep[k][:],
                                    hat[k][:, h * 2048 + m * 512:
                                           h * 2048 + (m + 1) * 512],
                                    start=(k == 0), stop=(k == 1),
                                )
                        lt = pre.tile([128, 2048], dt.float32, tag="lntmp",
                                      bufs=1, name=f"lts_{name}{h}")
                        nc.scalar.activation(lt[:], su[:], AF.Ln,
                                             bias=b4096[:], scale=-1.0)
                        nc.scalar.activation(sinv[:, hs], lt[:], AF.Exp,
                                             scale=-1.0)

                    # Su_cols [128, NJT] (j on partitions) -> 1/S exact
                    suc = pps.tile([128, NJT], dt.float32, tag="psbig",
                                   name=f"suc_{name}")
                    for jt in range(NJT):
                        for k in range(2):
                            nc.tensor.matmul(
                                suc[:, jt:jt + 1],
                                hat[k][:, jt * 128:(jt + 1) * 128],
                                u_bf[k][:],
                                start=(k == 0), stop=(k == 1),
                            )
                    scol = pre.tile([128, NJT], dt.float32, tag="scol", bufs=2,
                                    name=f"scol_{name}")
                    nc.vector.tensor_scalar(scol[:], suc[:], -1.0, float(HW),
                                            op0=AluOpType.mult,
                                            op1=AluOpType.add)
                    s_ic = pers.tile([128, NJT], dt.float32, tag=f"sic_{name}",
                                     name=f"sic_{name}")
                    nc.vector.reciprocal(s_ic[:], scol[:])
                    sic[name] = s_ic

                    # content lhsT = hat * Sinv * 2^16 in fp8 (negated for c
                    # so the content matmul sums x-part and c-part directly)
                    lc8 = LCX8 if name == "x" else LCC8
                    sgn = 65536.0 if name == "x" else -65536.0
                    for k in range(2):
                        nc.vector.scalar_tensor_tensor(
                            lc8[:, k, :], hat[k][:], sgn, sinv[:],
                            op0=AluOpType.mult, op1=AluOpType.mult)

                process("x", 0, "xc", HX)
                process("c", C, "xc", HC)
                process("s", 2 * C, "s", None)

                nc.vector.tensor_sub(dSc[:], sic["x"][:], sic["c"][:])

            # ---------------- main loop ----------------
            with tc.tile_pool(name="cps", bufs=2, space="PSUM") as cps, \
                 tc.tile_pool(name="sps", bufs=2, space="PSUM") as sps, \
                 tc.tile_pool(name="dmp", bufs=2) as dmp:
                for jt in range(NJT):
                    js = slice(jt * 128, (jt + 1) * 128)
                    for it in range(NIT):
                        idx = jt * NIT + it
                        # content: psG = 2^20 (x~^T x^q - c~^T c^q), fp8
                        # DoubleRow (K = 256 per instruction)
                        psG = cps.tile([128, IT], dt.float32, tag="psG",
                                       name=f"psG{idx}")
                        for m in range(2):
                            fs = slice(it * IT + m * 512,
                                       it * IT + (m + 1) * 512)
                            nc.tensor.matmul(
                                psG[:, m * 512:(m + 1) * 512],
                                LCX8[:, :, js], RX8[:, :, fs],
                                start=True, stop=False, perf_mode=PM,
                            )
                            nc.tensor.matmul(
                                psG[:, m * 512:(m + 1) * 512],
                                LCC8[:, :, js], RC8[:, :, fs],
                                start=False, stop=True, perf_mode=PM,
                            )
                        dump = dmp.tile([128, IT], dt.bfloat16, tag="adump",
                                        name=f"adump{idx}")
                        nc.scalar.activation(
                            dump[:], psG[:], AF.Abs,
                            bias=dSc[:, jt:jt + 1], scale=-(2.0 ** -20),
                            accum_out=csum_slots[:, idx:idx + 1],
                        )
                        # style: psS = 2^8 s^^T x^q, fp8 DoubleRow (K = 256)
                        psS = sps.tile([128, IT], dt.float32, tag="psS",
                                       name=f"psS{idx}")
                        for m in range(2):
                            fs = slice(it * IT + m * 512,
                                       it * IT + (m + 1) * 512)
                            nc.tensor.matmul(
                                psS[:, m * 512:(m + 1) * 512],
                                LS8[:, :, js], RX8[:, :, fs],
                                start=True, stop=True, perf_mode=PM,
                            )
                        nc.vector.reduce_max(m2slots[:, idx:idx + 1], psS[:],
                                             axis=AX.X)
                        sl = m1acc[:, it * IT:(it + 1) * IT]
                        if jt == 0:
                            nc.vector.tensor_copy(sl, psS[:])
                        else:
                            nc.vector.tensor_max(sl, sl, psS[:])

                # ---------------- finishers ----------------
                fin = dmp.tile([128, NJT + 1], dt.float32, tag="fin",
                               name="fin")
                m2v = m2slots[:].rearrange("p (j t) -> p j t", t=NIT)
                nc.vector.tensor_max(fin[:, 0:NJT], m2v[:, :, 0], m2v[:, :, 1])
                nc.vector.reduce_sum(fin[:, NJT:NJT + 1], csum_slots[:],
                                     axis=AX.X)
                nc.sync.dma_start(o_all[:, 0:NJT + 1], fin[:])
                nc.sync.dma_start(o_all[:, NJT + 1:], m1acc[:])

    nc.finalize()
    return nc


def _get_nc():
    global _CACHED_NC
    if _CACHED_NC is None:
        import os
        _CACHED_NC = _build(repeat=int(os.environ.get("KREPEAT", "1")))
    return _CACHED_NC


_RUNNER = None


def _get_runner():
    """Compile the 8-core PJRT executable once; returns run(in_maps)->results.

    Mirrors concourse.bass2jax.run_bass_via_pjrt but AOT-compiles with
    bass_effect suppressed (fast C++ dispatch) and caches the executable
    so repeated kernel() calls only pay device execution.
    """
    global _RUNNER
    if _RUNNER is not None:
        return _RUNNER
    import jax
    import numpy as _np
    from jax.sharding import Mesh, PartitionSpec
    from jax.experimental.shard_map import shard_map
    from concourse import mybir, bass2jax
    from concourse.bass2jax import (_bass_exec_p, partition_id_tensor,
                                    fast_dispatch_compile)

    bass2jax.install_neuronx_cc_hook()
    nc = _get_nc()
    partition_name = (nc.partition_id_tensor.name
                      if nc.partition_id_tensor else None)

    in_names, out_names, out_avals, zero_outs = [], [], [], []
    in_shapes = []
    for alloc in nc.m.functions[0].allocations:
        if not isinstance(alloc, mybir.MemoryLocationSet):
            continue
        name = alloc.memorylocations[0].name
        if alloc.kind == "ExternalInput":
            if name != partition_name:
                in_names.append(name)
                in_shapes.append((tuple(alloc.tensor_shape),
                                  mybir.dt.np(alloc.dtype)))
        elif alloc.kind == "ExternalOutput":
            out_names.append(name)
            shape = tuple(alloc.tensor_shape)
            dtype = mybir.dt.np(alloc.dtype)
            out_avals.append(jax.core.ShapedArray(shape, dtype))
            zero_outs.append(_np.zeros((NCORES * shape[0], *shape[1:]), dtype))
    n_params = len(in_names)
    n_outs = len(out_avals)
    all_names = list(in_names) + list(out_names)
    if partition_name is not None:
        all_names.append(partition_name)

    def _body(*args):
        operands = list(args)
        if partition_name is not None:
            operands.append(partition_id_tensor())
        return tuple(_bass_exec_p.bind(
            *operands,
            out_avals=tuple(out_avals),
            in_names=tuple(all_names),
            out_names=tuple(out_names),
            lowering_input_output_aliases=(),
            sim_require_finite=True,
            sim_require_nnan=True,
            nc=nc,
        ))

    devices = jax.devices()[:NCORES]
    mesh = Mesh(_np.asarray(devices), ("core",))
    from jax.sharding import NamedSharding
    sh = NamedSharding(mesh, PartitionSpec("core"))

    # AOT-compile with bass_effect suppressed: the default effectful path
    # forces slow Python dispatch with effect tokens on every call; the
    # fast path dispatches through C++ (see bass2jax.fast_dispatch_compile).
    arg_structs = (
        [jax.ShapeDtypeStruct((NCORES * s[0], *s[1:]), dt, sharding=sh)
         for s, dt in in_shapes]
        + [jax.ShapeDtypeStruct((NCORES * a.shape[0], *a.shape[1:]), a.dtype,
                                sharding=sh) for a in out_avals]
    )

    def _compile():
        return jax.jit(
            shard_map(_body, mesh=mesh,
                      in_specs=(PartitionSpec("core"),) * (n_params + n_outs),
                      out_specs=(PartitionSpec("core"),) * n_outs,
                      check_rep=False),
            keep_unused=True,
        ).lower(*arg_structs).compile()

    sharded = fast_dispatch_compile(_compile)
    zero_dev = [jax.device_put(a, sh) for a in zero_outs]

    def prepare(in_maps):
        """Stage concatenated inputs onto the devices once (for timing)."""
        concat_in = [
            _np.concatenate([in_maps[c][nm] for c in range(NCORES)], axis=0)
            for nm in in_names
        ]
        return [jax.device_put(a, sh) for a in concat_in]

    def exec_prepared(staged):
        out_arrs = sharded(*staged, *zero_dev)
        jax.block_until_ready(out_arrs)
        return out_arrs

    def exec_async(staged):
        """Dispatch one execution without blocking (pipelined timing)."""
        return sharded(*staged, *zero_dev)

    def run(in_maps):
        concat_in = [
            jax.device_put(
                _np.concatenate([in_maps[c][nm] for c in range(NCORES)],
                                axis=0), sh)
            for nm in in_names
        ]
        out_arrs = sharded(*concat_in, *zero_dev)
        jax.block_until_ready(out_arrs)
        return [
            {nm: _np.asarray(out_arrs[i]).reshape(NCORES, *out_avals[i].shape)[c]
             for i, nm in enumerate(out_names)}
            for c in range(NCORES)
        ]

    run.prepare = prepare
    run.exec_prepared = exec_prepared
    run.exec_async = exec_async
    _RUNNER = run
    return run


def _make_in_maps(x_feat, c_feat, s_feat):
    import ml_dtypes
    x = np.asarray(x_feat, dtype=np.float32).reshape(B, C, HW)
    c = np.asarray(c_feat, dtype=np.float32).reshape(B, C, HW)
    s = np.asarray(s_feat, dtype=np.float32).reshape(B, C, HW)
    in_maps = []
    for k in range(NCORES):
        b, ih = k // 2, k % 2
        parts = []
        for a in (x, c, s):
            ab = a[b]
            if ih:
                ab = np.roll(ab, -IQ, axis=1)
            parts.append(ab)
        xin = np.concatenate(parts, axis=0).astype(ml_dtypes.bfloat16)
        in_maps.append({"xin": np.ascontiguousarray(xin)})
    return in_maps


def kernel(x_feat, c_feat, s_feat):
    outs = _get_runner()(_make_in_maps(x_feat, c_feat, s_feat))

    total = sum(float(r["oall"][:, NJT].sum()) for r in outs)
    content = total / (B * HW)

    # style partials carry the fp8 scale 2^4 * 2^4 = 256
    m1total = sum(
        float(r["oall"][:, NJT + 1:].max(axis=0).sum()) for r in outs) / 256.0
    m1mean = 1.0 - m1total / (B * HW)
    m2mean = 0.0
    for b_ in range(B):
        flats = []
        for ih in range(2):
            dev = outs[2 * b_ + ih]["oall"][:, :NJT]  # [128 p, 32 jt]
            flat = dev.T.ravel() / 256.0  # index j_dev = jt*128 + p
            flats.append(np.roll(flat, IQ * ih))
        mx = np.maximum(flats[0], flats[1])
        m2mean += float((1.0 - mx).mean())
    m2mean /= B
    style = max(m1mean, m2mean)

    return (np.float32(content), np.float32(style))


# revision 22
# speedup vs baseline: 132.5992x; 1.0747x over previous
"""ContentStyleReltLoss kernel for 8 Trainium2 NeuronCores.

Sharding: core k handles (batch b = k//2, query-half ih = k%2).
Host rolls the HW columns of x/c/s by -ih*2048 so every core's 2048
"query" columns sit at device columns 0:2048 — all cores run identical
code. Each core computes, for its 2048 query columns i against ALL
4096 "key" columns j:

  content partial: sum_{j, i} | dS(j) - (x~_j . x^q_i - c~_j . c^q_i) |
     with x^ = x/||x|| per column, S_x(j) = HW - u_x . x^_j, u_x = sum_i x^_i,
     Sinv = 1/S, x~ = x^ * Sinv_x, dS(j) = Sinv_x(j) - Sinv_c(j)
  style partials: sim2(j, i) = s^_j . x^q_i
     m1sum = sum_i max_j sim2   (scalar, final per core)
     m2part(j) = max_{i in half} sim2  (host maxes the two i-halves)

Device layout: j on partitions (32 tiles of 128), i on free dim.
Single bf16 input [3C, HW]; single f32 output [128, 34]
(cols 0:32 m2part, col 32 content partial per j-partition, col 33 m1sum).
"""

import numpy as np

B, C, H, W = 4, 256, 64, 64
HW = H * W          # 4096
IQ = HW // 2        # 2048 query columns per core
NCORES = 8
NJT = HW // 128     # 32 j-tiles
NIT = 2             # i-tiles
IT = IQ // NIT      # 1024

_CACHED_NC = None


def _build(repeat=1):
    import concourse.bacc as bacc
    import concourse.tile as tile
    from concourse import mybir, bass_isa
    from concourse.alu_op_type import AluOpType
    from contextlib import ExitStack

    dt = mybir.dt
    AF = mybir.ActivationFunctionType
    AX = mybir.AxisListType

    nc = bacc.Bacc(None)

    xin = nc.dram_tensor("xin", [3 * C, HW], dt.bfloat16, kind="ExternalInput")
    # cols 0:NJT m2part, col NJT content partial, col NJT+1 m1 partial sums
    o_all = nc.dram_tensor("oall", [128, NJT + 2], dt.float32,
                           kind="ExternalOutput")

    F8 = dt.float8e4
    PM = mybir.MatmulPerfMode.DoubleRow

    with tile.TileContext(nc) as tc, ExitStack() as top:
        pers = top.enter_context(tc.tile_pool(name="pers", bufs=1))
        for _rep in range(repeat):
            # ---------------- persistent tiles ----------------
            # fp8 DoubleRow operands: dim1 = channel-half (K subtile).
            # content lhsT: x~ * 2^16  and  -c~ * 2^16
            LCX8 = pers.tile([128, 2, HW], F8, tag="lcx8", name="LCX8")
            LCC8 = pers.tile([128, 2, HW], F8, tag="lcc8", name="LCC8")
            # style lhsT: s^ * 2^4
            LS8 = pers.tile([128, 2, HW], F8, tag="ls8", name="LS8")
            # rhs (query cols): x^q * 2^4, c^q * 2^4
            RX8 = pers.tile([128, 2, IQ], F8, tag="rx8", name="RX8")
            RC8 = pers.tile([128, 2, IQ], F8, tag="rc8", name="RC8")
            # normalized columns bf16 (preprocessing operands)
            HX = [pers.tile([128, HW], dt.bfloat16, tag=f"hx{i}", name=f"HX{i}")
                  for i in range(2)]
            HC = [pers.tile([128, HW], dt.bfloat16, tag=f"hc{i}", name=f"HC{i}")
                  for i in range(2)]
            dSc = pers.tile([128, NJT], dt.float32, tag="dsc", name="dSc")
            csum_slots = pers.tile([128, NJT * NIT], dt.float32, tag="cslot",
                                   name="cslot")
            m2slots = pers.tile([128, NJT * NIT], dt.float32, tag="m2slot",
                                name="m2slot")
            m1acc = pers.tile([128, IQ], dt.bfloat16, tag="m1acc", name="m1acc")
            ones_bf = pers.tile([128, 128], dt.bfloat16, tag="ones",
                                name="ones_bf")
            nc.vector.memset(ones_bf[:], 1.0)
            b4096 = pers.tile([128, 1], dt.float32, tag="b4096", name="b4096")
            nc.vector.memset(b4096[:], float(HW))
            # identity (bf16) for PE transposes: 1 where free idx == partition
            it16 = pers.tile([128, 128], dt.int16, tag="it16", name="it16")
            nc.gpsimd.iota(it16[:], pattern=[[1, 128]], base=0,
                           channel_multiplier=-1)
            ident = pers.tile([128, 128], dt.bfloat16, tag="ident",
                              name="ident")
            nc.vector.tensor_scalar(ident[:], it16[:], 0.0, None,
                                    op0=AluOpType.is_equal)

            sic = {}  # per-column 1/S in j-partition layout, for x and c

            # ---------------- preprocessing ----------------
            with tc.tile_pool(name="pre", bufs=1) as pre, \
                 tc.tile_pool(name="pps", bufs=2, space="PSUM") as pps:

                def process(name, row0, kind, dst):
                    """Load rows [row0, row0+C) of xin, normalize columns.
                    kind 'xc': dst = 2 bf16 [128, HW] hat tiles;
                    kind 's': write s^ * 16 into LS8 (fp8) directly."""
                    raw = []
                    for k in range(2):
                        t = pre.tile([128, HW], dt.bfloat16, tag=f"raw{k}",
                                     bufs=2, name=f"raw_{name}{k}")
                        r0 = row0 + k * 128
                        nc.sync.dma_start(t[:], xin[r0:r0 + 128, :])
                        raw.append(t)
                    # column rnorm = 1/||col||, broadcast on partitions
                    rn = pre.tile([128, HW], dt.float32, tag="rn", bufs=1,
                                  name=f"rn_{name}")
                    for h in range(2):
                        hs = slice(h * 2048, (h + 1) * 2048)
                        sq = []
                        for k in range(2):
                            q = pre.tile([128, 2048], dt.bfloat16,
                                         tag=f"sqh{k}", bufs=1,
                                         name=f"sq_{name}{h}{k}")
                            nc.gpsimd.tensor_mul(q[:], raw[k][:, hs],
                                                 raw[k][:, hs])
                            sq.append(q)
                        ns = pps.tile([128, 2048], dt.float32, tag="psbig",
                                      name=f"ns_{name}{h}")
                        for k in range(2):
                            for m in range(4):
                                nc.tensor.matmul(
                                    ns[:, m * 512:(m + 1) * 512],
                                    ones_bf[:],
                                    sq[k][:, m * 512:(m + 1) * 512],
                                    start=(k == 0), stop=(k == 1),
                                )
                        lt = pre.tile([128, 2048], dt.float32, tag="lntmp",
                                      bufs=1, name=f"lt_{name}{h}")
                        nc.scalar.activation(lt[:], ns[:], AF.Ln)
                        nc.scalar.activation(rn[:, hs], lt[:], AF.Exp,
                                             scale=-0.5)
                    if kind == "s":
                        for k in range(2):
                            nc.vector.scalar_tensor_tensor(
                                LS8[:, k, :], raw[k][:], 16.0, rn[:],
                                op0=AluOpType.mult, op1=AluOpType.mult)
                        return
                    # normalized columns (bf16) into dst
                    for k in range(2):
                        nc.vector.tensor_mul(dst[k][:], raw[k][:], rn[:])

                    hat = dst
                    # fp8 rhs (query cols * 2^4) as soon as hat is ready
                    r8 = RX8 if name == "x" else RC8
                    for k in range(2):
                        nc.scalar.activation(r8[:, k, :], hat[k][:, 0:IQ],
                                             AF.Copy, scale=16.0)
                    # u = sum_i hat_i
                    u = [pre.tile([128, 1], dt.float32, tag=f"uu{k}", bufs=2,
                                  name=f"uu_{name}{k}") for k in range(2)]
                    u_bf = [pre.tile([128, 1], dt.bfloat16, tag=f"ub{k}",
                                     bufs=2, name=f"ub_{name}{k}")
                            for k in range(2)]
                    u_rep = [pre.tile([128, 128], dt.bfloat16, tag=f"ur{k}",
                                      bufs=2, name=f"ur_{name}{k}")
                             for k in range(2)]
                    for k in range(2):
                        nc.vector.reduce_sum(u[k][:], hat[k][:], axis=AX.X)
                        nc.vector.tensor_copy(u_bf[k][:], u[k][:])
                        nc.vector.tensor_scalar(u_rep[k][:], ones_bf[:],
                                                u[k][:], None,
                                                op0=AluOpType.mult)

                    # Sinv broadcast = exp(-ln(HW - u.hat_j))  [128, HW] f32
                    sinv = pre.tile([128, HW], dt.float32, tag="sinv", bufs=1,
                                    name=f"sinv_{name}")
                    for h in range(2):
                        hs = slice(h * 2048, (h + 1) * 2048)
                        su = pps.tile([128, 2048], dt.float32, tag="psbig",
                                      name=f"su_{name}{h}")
                        for k in range(2):
                            for m in range(4):
                                nc.tensor.matmul(
                                    su[:, m * 512:(m + 1) * 512],
                                    u_rep[k][:],
                                    hat[k][:, h * 2048 + m * 512:
                                           h * 2048 + (m + 1) * 512],
                                    start=(k == 0), stop=(k == 1),
                                )
                        lt = pre.tile([128, 2048], dt.float32, tag="lntmp",
                                      bufs=1, name=f"lts_{name}{h}")
                        nc.scalar.activation(lt[:], su[:], AF.Ln,
                                             bias=b4096[:], scale=-1.0)
                        nc.scalar.activation(sinv[:, hs], lt[:], AF.Exp,
                                             scale=-1.0)

                    # Su_cols [128, NJT] (j on partitions) -> 1/S exact
                    suc = pps.tile([128, NJT], dt.float32, tag="psbig",
                                   name=f"suc_{name}")
                    for jt in range(NJT):
                        for k in range(2):
                            nc.tensor.matmul(
                                suc[:, jt:jt + 1],
                                hat[k][:, jt * 128:(jt + 1) * 128],
                                u_bf[k][:],
                                start=(k == 0), stop=(k == 1),
                            )
                    scol = pre.tile([128, NJT], dt.float32, tag="scol", bufs=2,
                                    name=f"scol_{name}")
                    nc.vector.tensor_scalar(scol[:], suc[:], -1.0, float(HW),
                                            op0=AluOpType.mult,
                                            op1=AluOpType.add)
                    s_ic = pers.tile([128, NJT], dt.float32, tag=f"sic_{name}",
                                     name=f"sic_{name}")
                    nc.vector.reciprocal(s_ic[:], scol[:])
                    sic[name] = s_ic

                    # content lhsT = hat * Sinv * 2^16 in fp8 (negated for c
                    # so the content matmul sums x-part and c-part directly)
                    lc8 = LCX8 if name == "x" else LCC8
                    sgn = 65536.0 if name == "x" else -65536.0
                    for k in range(2):
                        nc.vector.scalar_tensor_tensor(
                            lc8[:, k, :], hat[k][:], sgn, sinv[:],
                            op0=AluOpType.mult, op1=AluOpType.mult)

                process("x", 0, "xc", HX)
                process("c", C, "xc", HC)
                process("s", 2 * C, "s", None)

                nc.vector.tensor_sub(dSc[:], sic["x"][:], sic["c"][:])

            # ---------------- main loop ----------------
            with tc.tile_pool(name="cps", bufs=2, space="PSUM") as cps, \
                 tc.tile_pool(name="sps", bufs=2, space="PSUM") as sps, \
                 tc.tile_pool(name="dmp", bufs=2) as dmp:
                for jt in range(NJT):
                    js = slice(jt * 128, (jt + 1) * 128)
                    for it in range(NIT):
                        idx = jt * NIT + it
                        # content: psG = 2^20 (x~^T x^q - c~^T c^q), fp8
                        # DoubleRow (K = 256 per instruction)
                        psG = cps.tile([128, IT], dt.float32, tag="psG",
                                       name=f"psG{idx}")
                        for m in range(2):
                            fs = slice(it * IT + m * 512,
                                       it * IT + (m + 1) * 512)
                            nc.tensor.matmul(
                                psG[:, m * 512:(m + 1) * 512],
                                LCX8[:, :, js], RX8[:, :, fs],
                                start=True, stop=False, perf_mode=PM,
                            )
                            nc.tensor.matmul(
                                psG[:, m * 512:(m + 1) * 512],
                                LCC8[:, :, js], RC8[:, :, fs],
                                start=False, stop=True, perf_mode=PM,
                            )
                        dump = dmp.tile([128, IT], dt.bfloat16, tag="adump",
                                        name=f"adump{idx}")
                        nc.scalar.activation(
                            dump[:], psG[:], AF.Abs,
                            bias=dSc[:, jt:jt + 1], scale=-(2.0 ** -20),
                            accum_out=csum_slots[:, idx:idx + 1],
                        )
                        # style: psS = 2^8 s^^T x^q, fp8 DoubleRow (K = 256)
                        psS = sps.tile([128, IT], dt.float32, tag="psS",
                                       name=f"psS{idx}")
                        for m in range(2):
                            fs = slice(it * IT + m * 512,
                                       it * IT + (m + 1) * 512)
                            nc.tensor.matmul(
                                psS[:, m * 512:(m + 1) * 512],
                                LS8[:, :, js], RX8[:, :, fs],
                                start=True, stop=True, perf_mode=PM,
                            )
                        nc.vector.reduce_max(m2slots[:, idx:idx + 1], psS[:],
                                             axis=AX.X)
                        sl = m1acc[:, it * IT:(it + 1) * IT]
                        if jt == 0:
                            nc.vector.tensor_copy(sl, psS[:])
                        else:
                            nc.vector.tensor_max(sl, sl, psS[:])

            # ---------------- finishers ----------------
            with tc.tile_pool(name="fino", bufs=1) as dmp:
                fin = dmp.tile([128, NJT + 2], dt.float32, tag="fin",
                               name="fin")
                m2v = m2slots[:].rearrange("p (j t) -> p j t", t=NIT)
                nc.vector.tensor_max(fin[:, 0:NJT], m2v[:, :, 0], m2v[:, :, 1])
                nc.vector.reduce_sum(fin[:, NJT:NJT + 1], csum_slots[:],
                                     axis=AX.X)
                # m1: max over partitions via PE transposes, then per-partition
                # partial sums (host adds the 128 values)
                mt = dmp.tile([128, IQ // 128], dt.float32, tag="mt",
                              name="mt")
                with tc.tile_pool(name="tps", bufs=2, space="PSUM") as tps:
                    for cb in range(IQ // 128):
                        psT = tps.tile([128, 128], dt.bfloat16, tag="psT",
                                       name=f"psT{cb}")
                        nc.tensor.transpose(
                            psT[:], m1acc[:, cb * 128:(cb + 1) * 128],
                            ident[:])
                        nc.vector.reduce_max(mt[:, cb:cb + 1], psT[:],
                                             axis=AX.X)
                nc.vector.reduce_sum(fin[:, NJT + 1:NJT + 2], mt[:],
                                     axis=AX.X)
                nc.sync.dma_start(o_all[:], fin[:])

    nc.finalize()
    return nc


def _get_nc():
    global _CACHED_NC
    if _CACHED_NC is None:
        import os
        _CACHED_NC = _build(repeat=int(os.environ.get("KREPEAT", "1")))
    return _CACHED_NC


_RUNNER = None


def _get_runner():
    """Compile the 8-core PJRT executable once; returns run(in_maps)->results.

    Mirrors concourse.bass2jax.run_bass_via_pjrt but AOT-compiles with
    bass_effect suppressed (fast C++ dispatch) and caches the executable
    so repeated kernel() calls only pay device execution.
    """
    global _RUNNER
    if _RUNNER is not None:
        return _RUNNER
    import jax
    import numpy as _np
    from jax.sharding import Mesh, PartitionSpec
    from jax.experimental.shard_map import shard_map
    from concourse import mybir, bass2jax
    from concourse.bass2jax import (_bass_exec_p, partition_id_tensor,
                                    fast_dispatch_compile)

    bass2jax.install_neuronx_cc_hook()
    nc = _get_nc()
    partition_name = (nc.partition_id_tensor.name
                      if nc.partition_id_tensor else None)

    in_names, out_names, out_avals, zero_outs = [], [], [], []
    in_shapes = []
    for alloc in nc.m.functions[0].allocations:
        if not isinstance(alloc, mybir.MemoryLocationSet):
            continue
        name = alloc.memorylocations[0].name
        if alloc.kind == "ExternalInput":
            if name != partition_name:
                in_names.append(name)
                in_shapes.append((tuple(alloc.tensor_shape),
                                  mybir.dt.np(alloc.dtype)))
        elif alloc.kind == "ExternalOutput":
            out_names.append(name)
            shape = tuple(alloc.tensor_shape)
            dtype = mybir.dt.np(alloc.dtype)
            out_avals.append(jax.core.ShapedArray(shape, dtype))
            zero_outs.append(_np.zeros((NCORES * shape[0], *shape[1:]), dtype))
    n_params = len(in_names)
    n_outs = len(out_avals)
    all_names = list(in_names) + list(out_names)
    if partition_name is not None:
        all_names.append(partition_name)

    def _body(*args):
        operands = list(args)
        if partition_name is not None:
            operands.append(partition_id_tensor())
        return tuple(_bass_exec_p.bind(
            *operands,
            out_avals=tuple(out_avals),
            in_names=tuple(all_names),
            out_names=tuple(out_names),
            lowering_input_output_aliases=(),
            sim_require_finite=True,
            sim_require_nnan=True,
            nc=nc,
        ))

    devices = jax.devices()[:NCORES]
    mesh = Mesh(_np.asarray(devices), ("core",))
    from jax.sharding import NamedSharding
    sh = NamedSharding(mesh, PartitionSpec("core"))

    # AOT-compile with bass_effect suppressed: the default effectful path
    # forces slow Python dispatch with effect tokens on every call; the
    # fast path dispatches through C++ (see bass2jax.fast_dispatch_compile).
    arg_structs = (
        [jax.ShapeDtypeStruct((NCORES * s[0], *s[1:]), dt, sharding=sh)
         for s, dt in in_shapes]
        + [jax.ShapeDtypeStruct((NCORES * a.shape[0], *a.shape[1:]), a.dtype,
                                sharding=sh) for a in out_avals]
    )

    def _compile():
        return jax.jit(
            shard_map(_body, mesh=mesh,
                      in_specs=(PartitionSpec("core"),) * (n_params + n_outs),
                      out_specs=(PartitionSpec("core"),) * n_outs,
                      check_rep=False),
            keep_unused=True,
        ).lower(*arg_structs).compile()

    sharded = fast_dispatch_compile(_compile)
    zero_dev = [jax.device_put(a, sh) for a in zero_outs]

    def prepare(in_maps):
        """Stage concatenated inputs onto the devices once (for timing)."""
        concat_in = [
            _np.concatenate([in_maps[c][nm] for c in range(NCORES)], axis=0)
            for nm in in_names
        ]
        return [jax.device_put(a, sh) for a in concat_in]

    def exec_prepared(staged):
        out_arrs = sharded(*staged, *zero_dev)
        jax.block_until_ready(out_arrs)
        return out_arrs

    def exec_async(staged):
        """Dispatch one execution without blocking (pipelined timing)."""
        return sharded(*staged, *zero_dev)

    def run(in_maps):
        concat_in = [
            jax.device_put(
                _np.concatenate([in_maps[c][nm] for c in range(NCORES)],
                                axis=0), sh)
            for nm in in_names
        ]
        out_arrs = sharded(*concat_in, *zero_dev)
        jax.block_until_ready(out_arrs)
        return [
            {nm: _np.asarray(out_arrs[i]).reshape(NCORES, *out_avals[i].shape)[c]
             for i, nm in enumerate(out_names)}
            for c in range(NCORES)
        ]

    run.prepare = prepare
    run.exec_prepared = exec_prepared
    run.exec_async = exec_async
    _RUNNER = run
    return run


def _make_in_maps(x_feat, c_feat, s_feat):
    import ml_dtypes
    x = np.asarray(x_feat, dtype=np.float32).reshape(B, C, HW)
    c = np.asarray(c_feat, dtype=np.float32).reshape(B, C, HW)
    s = np.asarray(s_feat, dtype=np.float32).reshape(B, C, HW)
    in_maps = []
    for k in range(NCORES):
        b, ih = k // 2, k % 2
        parts = []
        for a in (x, c, s):
            ab = a[b]
            if ih:
                ab = np.roll(ab, -IQ, axis=1)
            parts.append(ab)
        xin = np.concatenate(parts, axis=0).astype(ml_dtypes.bfloat16)
        in_maps.append({"xin": np.ascontiguousarray(xin)})
    return in_maps


def kernel(x_feat, c_feat, s_feat):
    outs = _get_runner()(_make_in_maps(x_feat, c_feat, s_feat))

    total = sum(float(r["oall"][:, NJT].sum()) for r in outs)
    content = total / (B * HW)

    # style partials carry the fp8 scale 2^4 * 2^4 = 256
    m1total = sum(float(r["oall"][:, NJT + 1].sum()) for r in outs) / 256.0
    m1mean = 1.0 - m1total / (B * HW)
    m2mean = 0.0
    for b_ in range(B):
        flats = []
        for ih in range(2):
            dev = outs[2 * b_ + ih]["oall"][:, :NJT]  # [128 p, 32 jt]
            flat = dev.T.ravel() / 256.0  # index j_dev = jt*128 + p
            flats.append(np.roll(flat, IQ * ih))
        mx = np.maximum(flats[0], flats[1])
        m2mean += float((1.0 - mx).mean())
    m2mean /= B
    style = max(m1mean, m2mean)

    return (np.float32(content), np.float32(style))


# revision 26
# speedup vs baseline: 135.1989x; 1.0196x over previous
"""ContentStyleReltLoss kernel for 8 Trainium2 NeuronCores.

Sharding: core k handles (batch b = k//2, query-half ih = k%2).
Host rolls the HW columns of x/c/s by -ih*2048 so every core's 2048
"query" columns sit at device columns 0:2048 — all cores run identical
code. Each core computes, for its 2048 query columns i against ALL
4096 "key" columns j:

  content partial: sum_{j, i} | dS(j) - (x~_j . x^q_i - c~_j . c^q_i) |
     with x^ = x/||x|| per column, S_x(j) = HW - u_x . x^_j, u_x = sum_i x^_i,
     Sinv = 1/S, x~ = x^ * Sinv_x, dS(j) = Sinv_x(j) - Sinv_c(j)
  style partials: sim2(j, i) = s^_j . x^q_i
     m1sum = sum_i max_j sim2   (scalar, final per core)
     m2part(j) = max_{i in half} sim2  (host maxes the two i-halves)

Device layout: j on partitions (32 tiles of 128), i on free dim.
Single bf16 input [3C, HW]; single f32 output [128, 34]
(cols 0:32 m2part, col 32 content partial per j-partition, col 33 m1sum).
"""

import numpy as np

B, C, H, W = 4, 256, 64, 64
HW = H * W          # 4096
IQ = HW // 2        # 2048 query columns per core
NCORES = 8
NJT = HW // 128     # 32 j-tiles
NIT = 2             # i-tiles
IT = IQ // NIT      # 1024

_CACHED_NC = None


def _build(repeat=1):
    import concourse.bacc as bacc
    import concourse.tile as tile
    from concourse import mybir, bass_isa
    from concourse.alu_op_type import AluOpType
    from contextlib import ExitStack

    dt = mybir.dt
    AF = mybir.ActivationFunctionType
    AX = mybir.AxisListType

    nc = bacc.Bacc(None)

    xin = nc.dram_tensor("xin", [3 * C, HW], dt.bfloat16, kind="ExternalInput")
    # cols 0:NJT m2part, col NJT content partial, col NJT+1 m1 partial sums
    o_all = nc.dram_tensor("oall", [128, NJT + 2], dt.float32,
                           kind="ExternalOutput")

    F8 = dt.float8e4
    PM = mybir.MatmulPerfMode.DoubleRow

    with tile.TileContext(nc) as tc, ExitStack() as top:
        pers = top.enter_context(tc.tile_pool(name="pers", bufs=1))
        for _rep in range(repeat):
            # ---------------- persistent tiles ----------------
            # fp8 DoubleRow operands: dim1 = channel-half (K subtile).
            # content lhsT: x~ * 2^16  and  -c~ * 2^16
            LCX8 = pers.tile([128, 2, HW], F8, tag="lcx8", name="LCX8")
            LCC8 = pers.tile([128, 2, HW], F8, tag="lcc8", name="LCC8")
            # style lhsT: s^ * 2^4
            LS8 = pers.tile([128, 2, HW], F8, tag="ls8", name="LS8")
            # rhs (query cols): x^q * 2^4, c^q * 2^4
            RX8 = pers.tile([128, 2, IQ], F8, tag="rx8", name="RX8")
            RC8 = pers.tile([128, 2, IQ], F8, tag="rc8", name="RC8")
            # normalized columns bf16 (preprocessing operands)
            HX = [pers.tile([128, HW], dt.bfloat16, tag=f"hx{i}", name=f"HX{i}")
                  for i in range(2)]
            HC = [pers.tile([128, HW], dt.bfloat16, tag=f"hc{i}", name=f"HC{i}")
                  for i in range(2)]
            dSc = pers.tile([128, NJT], dt.float32, tag="dsc", name="dSc")
            csum_slots = pers.tile([128, NJT * NIT], dt.float32, tag="cslot",
                                   name="cslot")
            m2slots = pers.tile([128, NJT * NIT], dt.float32, tag="m2slot",
                                name="m2slot")
            m1acc = pers.tile([128, IQ], dt.bfloat16, tag="m1acc", name="m1acc")
            ones_bf = pers.tile([128, 128], dt.bfloat16, tag="ones",
                                name="ones_bf")
            nc.vector.memset(ones_bf[:], 1.0)
            b4096 = pers.tile([128, 1], dt.float32, tag="b4096", name="b4096")
            nc.vector.memset(b4096[:], float(HW))
            # identity (bf16) for PE transposes: 1 where free idx == partition
            it16 = pers.tile([128, 128], dt.int16, tag="it16", name="it16")
            nc.gpsimd.iota(it16[:], pattern=[[1, 128]], base=0,
                           channel_multiplier=-1)
            ident = pers.tile([128, 128], dt.bfloat16, tag="ident",
                              name="ident")
            nc.vector.tensor_scalar(ident[:], it16[:], 0.0, None,
                                    op0=AluOpType.is_equal)

            sic = {}  # per-column 1/S in j-partition layout, for x and c

            # ---------------- preprocessing ----------------
            with tc.tile_pool(name="pre", bufs=1) as pre, \
                 tc.tile_pool(name="pps", bufs=2, space="PSUM") as pps:

                def process(name, row0, kind, dst):
                    """Load rows [row0, row0+C) of xin, normalize columns.
                    kind 'xc': dst = 2 bf16 [128, HW] hat tiles;
                    kind 's': write s^ * 16 into LS8 (fp8) directly."""
                    raw = []
                    for k in range(2):
                        t = pre.tile([128, HW], dt.bfloat16, tag=f"raw{k}",
                                     bufs=2, name=f"raw_{name}{k}")
                        r0 = row0 + k * 128
                        nc.sync.dma_start(t[:], xin[r0:r0 + 128, :])
                        raw.append(t)
                    # column rnorm = 1/||col||, broadcast on partitions
                    rn = pre.tile([128, HW], dt.float32, tag="rn", bufs=1,
                                  name=f"rn_{name}")
                    for h in range(2):
                        hs = slice(h * 2048, (h + 1) * 2048)
                        sq = []
                        for k in range(2):
                            q = pre.tile([128, 2048], dt.bfloat16,
                                         tag=f"sqh{k}", bufs=1,
                                         name=f"sq_{name}{h}{k}")
                            nc.gpsimd.tensor_mul(q[:], raw[k][:, hs],
                                                 raw[k][:, hs])
                            sq.append(q)
                        ns = pps.tile([128, 2048], dt.float32, tag="psbig",
                                      name=f"ns_{name}{h}")
                        for k in range(2):
                            for m in range(4):
                                nc.tensor.matmul(
                                    ns[:, m * 512:(m + 1) * 512],
                                    ones_bf[:],
                                    sq[k][:, m * 512:(m + 1) * 512],
                                    start=(k == 0), stop=(k == 1),
                                )
                        lt = pre.tile([128, 2048], dt.float32, tag="lntmp",
                                      bufs=1, name=f"lt_{name}{h}")
                        nc.scalar.activation(lt[:], ns[:], AF.Ln)
                        nc.scalar.activation(rn[:, hs], lt[:], AF.Exp,
                                             scale=-0.5)
                    if kind == "s":
                        for k in range(2):
                            nc.vector.scalar_tensor_tensor(
                                LS8[:, k, :], raw[k][:], 16.0, rn[:],
                                op0=AluOpType.mult, op1=AluOpType.mult)
                        return
                    # normalized columns (bf16) into dst
                    for k in range(2):
                        nc.vector.tensor_mul(dst[k][:], raw[k][:], rn[:])

                    hat = dst
                    # fp8 rhs (query cols * 2^4) as soon as hat is ready
                    r8 = RX8 if name == "x" else RC8
                    for k in range(2):
                        nc.scalar.activation(r8[:, k, :], hat[k][:, 0:IQ],
                                             AF.Copy, scale=16.0)
                    # u = sum_i hat_i
                    u = [pre.tile([128, 1], dt.float32, tag=f"uu{k}", bufs=2,
                                  name=f"uu_{name}{k}") for k in range(2)]
                    u_bf = [pre.tile([128, 1], dt.bfloat16, tag=f"ub{k}",
                                     bufs=2, name=f"ub_{name}{k}")
                            for k in range(2)]
                    u_rep = [pre.tile([128, 128], dt.bfloat16, tag=f"ur{k}",
                                      bufs=2, name=f"ur_{name}{k}")
                             for k in range(2)]
                    for k in range(2):
                        nc.vector.reduce_sum(u[k][:], hat[k][:], axis=AX.X)
                        nc.vector.tensor_copy(u_bf[k][:], u[k][:])
                        nc.vector.tensor_scalar(u_rep[k][:], ones_bf[:],
                                                u[k][:], None,
                                                op0=AluOpType.mult)

                    # Sinv broadcast = exp(-ln(HW - u.hat_j))  [128, HW] f32
                    sinv = pre.tile([128, HW], dt.float32, tag="sinv", bufs=1,
                                    name=f"sinv_{name}")
                    for h in range(2):
                        hs = slice(h * 2048, (h + 1) * 2048)
                        su = pps.tile([128, 2048], dt.float32, tag="psbig",
                                      name=f"su_{name}{h}")
                        for k in range(2):
                            for m in range(4):
                                nc.tensor.matmul(
                                    su[:, m * 512:(m + 1) * 512],
                                    u_rep[k][:],
                                    hat[k][:, h * 2048 + m * 512:
                                           h * 2048 + (m + 1) * 512],
                                    start=(k == 0), stop=(k == 1),
                                )
                        lt = pre.tile([128, 2048], dt.float32, tag="lntmp",
                                      bufs=1, name=f"lts_{name}{h}")
                        nc.scalar.activation(lt[:], su[:], AF.Ln,
                                             bias=b4096[:], scale=-1.0)
                        nc.scalar.activation(sinv[:, hs], lt[:], AF.Exp,
                                             scale=-1.0)

                    # Su_cols [128, NJT] (j on partitions) -> 1/S exact
                    suc = pps.tile([128, NJT], dt.float32, tag="psbig",
                                   name=f"suc_{name}")
                    for jt in range(NJT):
                        for k in range(2):
                            nc.tensor.matmul(
                                suc[:, jt:jt + 1],
                                hat[k][:, jt * 128:(jt + 1) * 128],
                                u_bf[k][:],
                                start=(k == 0), stop=(k == 1),
                            )
                    scol = pre.tile([128, NJT], dt.float32, tag="scol", bufs=2,
                                    name=f"scol_{name}")
                    nc.vector.tensor_scalar(scol[:], suc[:], -1.0, float(HW),
                                            op0=AluOpType.mult,
                                            op1=AluOpType.add)
                    s_ic = pers.tile([128, NJT], dt.float32, tag=f"sic_{name}",
                                     name=f"sic_{name}")
                    nc.vector.reciprocal(s_ic[:], scol[:])
                    sic[name] = s_ic

                    # content lhsT = hat * Sinv * 2^16 in fp8 (negated for c
                    # so the content matmul sums x-part and c-part directly)
                    lc8 = LCX8 if name == "x" else LCC8
                    sgn = 65536.0 if name == "x" else -65536.0
                    for k in range(2):
                        nc.vector.scalar_tensor_tensor(
                            lc8[:, k, :], hat[k][:], sgn, sinv[:],
                            op0=AluOpType.mult, op1=AluOpType.mult)

                process("x", 0, "xc", HX)
                process("c", C, "xc", HC)
                process("s", 2 * C, "s", None)

                nc.vector.tensor_sub(dSc[:], sic["x"][:], sic["c"][:])

            # ---------------- main loop ----------------
            with tc.tile_pool(name="cps", bufs=2, space="PSUM") as cps, \
                 tc.tile_pool(name="sps", bufs=2, space="PSUM") as sps, \
                 tc.tile_pool(name="dmp", bufs=2) as dmp:
                for jt in range(NJT):
                    js = slice(jt * 128, (jt + 1) * 128)
                    for it in range(NIT):
                        idx = jt * NIT + it
                        # content: psG = 2^20 (x~^T x^q - c~^T c^q), fp8
                        # DoubleRow (K = 256 per instruction)
                        psG = cps.tile([128, IT], dt.float32, tag="psG",
                                       name=f"psG{idx}")
                        for m in range(2):
                            fs = slice(it * IT + m * 512,
                                       it * IT + (m + 1) * 512)
                            nc.tensor.matmul(
                                psG[:, m * 512:(m + 1) * 512],
                                LCX8[:, :, js], RX8[:, :, fs],
                                start=True, stop=False, perf_mode=PM,
                            )
                            nc.tensor.matmul(
                                psG[:, m * 512:(m + 1) * 512],
                                LCC8[:, :, js], RC8[:, :, fs],
                                start=False, stop=True, perf_mode=PM,
                            )
                        dump = dmp.tile([128, IT], dt.bfloat16, tag="adump",
                                        name=f"adump{idx}")
                        nc.scalar.activation(
                            dump[:], psG[:], AF.Abs,
                            bias=dSc[:, jt:jt + 1], scale=-(2.0 ** -20),
                            accum_out=csum_slots[:, idx:idx + 1],
                        )
                        # style: psS = 2^8 s^^T x^q, fp8 DoubleRow (K = 256)
                        psS = sps.tile([128, IT], dt.float32, tag="psS",
                                       name=f"psS{idx}")
                        for m in range(2):
                            fs = slice(it * IT + m * 512,
                                       it * IT + (m + 1) * 512)
                            nc.tensor.matmul(
                                psS[:, m * 512:(m + 1) * 512],
                                LS8[:, :, js], RX8[:, :, fs],
                                start=True, stop=True, perf_mode=PM,
                            )
                        nc.vector.reduce_max(m2slots[:, idx:idx + 1], psS[:],
                                             axis=AX.X)
                        sl = m1acc[:, it * IT:(it + 1) * IT]
                        if jt == 0:
                            nc.vector.tensor_copy(sl, psS[:])
                        else:
                            nc.vector.tensor_max(sl, sl, psS[:])

            # ---------------- finishers ----------------
            with tc.tile_pool(name="fino", bufs=1) as dmp:
                fin = dmp.tile([128, NJT + 2], dt.float32, tag="fin",
                               name="fin")
                m2v = m2slots[:].rearrange("p (j t) -> p j t", t=NIT)
                nc.vector.tensor_max(fin[:, 0:NJT], m2v[:, :, 0], m2v[:, :, 1])
                nc.vector.reduce_sum(fin[:, NJT:NJT + 1], csum_slots[:],
                                     axis=AX.X)
                # m1: max over partitions via PE transposes, then per-partition
                # partial sums (host adds the 128 values)
                mt = dmp.tile([128, IQ // 128], dt.float32, tag="mt",
                              name="mt")
                with tc.tile_pool(name="tps", bufs=2, space="PSUM") as tps:
                    for cb in range(IQ // 128):
                        psT = tps.tile([128, 128], dt.bfloat16, tag="psT",
                                       name=f"psT{cb}")
                        nc.tensor.transpose(
                            psT[:], m1acc[:, cb * 128:(cb + 1) * 128],
                            ident[:])
                        nc.vector.reduce_max(mt[:, cb:cb + 1], psT[:],
                                             axis=AX.X)
                nc.vector.reduce_sum(fin[:, NJT + 1:NJT + 2], mt[:],
                                     axis=AX.X)
                nc.sync.dma_start(o_all[:], fin[:])

    nc.finalize()
    return nc


def _get_nc():
    global _CACHED_NC
    if _CACHED_NC is None:
        import os
        _CACHED_NC = _build(repeat=int(os.environ.get("KREPEAT", "1")))
    return _CACHED_NC


_RUNNER = None


def _get_runner():
    """Compile the 8-core PJRT executable once; returns run(in_maps)->results.

    Mirrors concourse.bass2jax.run_bass_via_pjrt but AOT-compiles with
    bass_effect suppressed (fast C++ dispatch) and caches the executable
    so repeated kernel() calls only pay device execution.
    """
    global _RUNNER
    if _RUNNER is not None:
        return _RUNNER
    import jax
    import numpy as _np
    from jax.sharding import Mesh, PartitionSpec
    from jax.experimental.shard_map import shard_map
    from concourse import mybir, bass2jax
    from concourse.bass2jax import (_bass_exec_p, partition_id_tensor,
                                    fast_dispatch_compile)

    bass2jax.install_neuronx_cc_hook()
    nc = _get_nc()
    partition_name = (nc.partition_id_tensor.name
                      if nc.partition_id_tensor else None)

    in_names, out_names, out_avals, zero_outs = [], [], [], []
    in_shapes = []
    for alloc in nc.m.functions[0].allocations:
        if not isinstance(alloc, mybir.MemoryLocationSet):
            continue
        name = alloc.memorylocations[0].name
        if alloc.kind == "ExternalInput":
            if name != partition_name:
                in_names.append(name)
                in_shapes.append((tuple(alloc.tensor_shape),
                                  mybir.dt.np(alloc.dtype)))
        elif alloc.kind == "ExternalOutput":
            out_names.append(name)
            shape = tuple(alloc.tensor_shape)
            dtype = mybir.dt.np(alloc.dtype)
            out_avals.append(jax.core.ShapedArray(shape, dtype))
            zero_outs.append(_np.zeros((NCORES * shape[0], *shape[1:]), dtype))
    n_params = len(in_names)
    n_outs = len(out_avals)
    all_names = list(in_names) + list(out_names)
    if partition_name is not None:
        all_names.append(partition_name)

    def _body(*args):
        operands = list(args)
        if partition_name is not None:
            operands.append(partition_id_tensor())
        return tuple(_bass_exec_p.bind(
            *operands,
            out_avals=tuple(out_avals),
            in_names=tuple(all_names),
            out_names=tuple(out_names),
            lowering_input_output_aliases=(),
            sim_require_finite=True,
            sim_require_nnan=True,
            nc=nc,
        ))

    devices = jax.devices()[:NCORES]
    mesh = Mesh(_np.asarray(devices), ("core",))
    from jax.sharding import NamedSharding
    sh = NamedSharding(mesh, PartitionSpec("core"))

    # AOT-compile with bass_effect suppressed: the default effectful path
    # forces slow Python dispatch with effect tokens on every call; the
    # fast path dispatches through C++ (see bass2jax.fast_dispatch_compile).
    arg_structs = (
        [jax.ShapeDtypeStruct((NCORES * s[0], *s[1:]), dt, sharding=sh)
         for s, dt in in_shapes]
        + [jax.ShapeDtypeStruct((NCORES * a.shape[0], *a.shape[1:]), a.dtype,
                                sharding=sh) for a in out_avals]
    )

    def _compile():
        return jax.jit(
            shard_map(_body, mesh=mesh,
                      in_specs=(PartitionSpec("core"),) * (n_params + n_outs),
                      out_specs=(PartitionSpec("core"),) * n_outs,
                      check_rep=False),
            keep_unused=True,
        ).lower(*arg_structs).compile()

    sharded = fast_dispatch_compile(_compile)
    zero_dev = [jax.device_put(a, sh) for a in zero_outs]

    def prepare(in_maps):
        """Stage concatenated inputs onto the devices once (for timing)."""
        concat_in = [
            _np.concatenate([in_maps[c][nm] for c in range(NCORES)], axis=0)
            for nm in in_names
        ]
        return [jax.device_put(a, sh) for a in concat_in]

    def exec_prepared(staged):
        out_arrs = sharded(*staged, *zero_dev)
        jax.block_until_ready(out_arrs)
        return out_arrs

    def exec_async(staged):
        """Dispatch one execution without blocking (pipelined timing)."""
        return sharded(*staged, *zero_dev)

    def run(in_maps):
        concat_in = [
            jax.device_put(
                _np.concatenate([in_maps[c][nm] for c in range(NCORES)],
                                axis=0), sh)
            for nm in in_names
        ]
        out_arrs = sharded(*concat_in, *zero_dev)
        jax.block_until_ready(out_arrs)
        return [
            {nm: _np.asarray(out_arrs[i]).reshape(NCORES, *out_avals[i].shape)[c]
             for i, nm in enumerate(out_names)}
            for c in range(NCORES)
        ]

    run.prepare = prepare
    run.exec_prepared = exec_prepared
    run.exec_async = exec_async
    _RUNNER = run
    return run


def _make_in_maps(x_feat, c_feat, s_feat):
    import ml_dtypes
    x = np.asarray(x_feat, dtype=np.float32).reshape(B, C, HW)
    c = np.asarray(c_feat, dtype=np.float32).reshape(B, C, HW)
    s = np.asarray(s_feat, dtype=np.float32).reshape(B, C, HW)
    in_maps = []
    for k in range(NCORES):
        b, ih = k // 2, k % 2
        parts = []
        for a in (x, c, s):
            ab = a[b]
            if ih:
                ab = np.roll(ab, -IQ, axis=1)
            parts.append(ab)
        xin = np.concatenate(parts, axis=0).astype(ml_dtypes.bfloat16)
        in_maps.append({"xin": np.ascontiguousarray(xin)})
    return in_maps


def kernel(x_feat, c_feat, s_feat):
    outs = _get_runner()(_make_in_maps(x_feat, c_feat, s_feat))

    total = sum(float(r["oall"][:, NJT].sum()) for r in outs)
    content = total / (B * HW)

    # style partials carry the fp8 scale 2^4 * 2^4 = 256
    m1total = sum(float(r["oall"][:, NJT + 1].sum()) for r in outs) / 256.0
    m1mean = 1.0 - m1total / (B * HW)
    m2mean = 0.0
    for b_ in range(B):
        flats = []
        for ih in range(2):
            dev = outs[2 * b_ + ih]["oall"][:, :NJT]  # [128 p, 32 jt]
            flat = dev.T.ravel() / 256.0  # index j_dev = jt*128 + p
            flats.append(np.roll(flat, IQ * ih))
        mx = np.maximum(flats[0], flats[1])
        m2mean += float((1.0 - mx).mean())
    m2mean /= B
    style = max(m1mean, m2mean)

    return (np.float32(content), np.float32(style))
